# revision 42
# baseline (speedup 1.0000x reference)
"""Bass/Trainium2 kernel for a 2-layer multi-head GAT (DocRE model).

Contract: kernel(**inputs) takes the FULL unsharded inputs as numpy arrays
and returns the FULL [512, 768] float32 output.

Sharding / dataflow (v2, collective-light):
- Layer 0 is row-sharded: each core streams its 64 rows of the edge tensor e
  once (block-major fp8-e3m4 layout) and computes BOTH layers' edge scores
  s_e = e @ V in a single pass (V holds 12 layer-0 head columns plus 16
  layer-1 "slot" columns = (dst core, slot) pairs).
- Layer-1 edge scores are AllToAll'd to a head-sharded layout DURING the
  e-stream (2 chunks; the first is hidden under the stream). Each core then
  owns 2 layer-1 heads: head A = core id (weight 1), head B = 8 + core//2
  (computed by two cores, each weighted 1/2).
- x1 is AllGathered once (small); h1 columns for the core's two heads are
  computed locally from the gathered x1 -> NO AllGather of the big h1.
- Per-head partial outputs (all 512 rows) are ReduceScatter'd (fp32 add) to
  give each core its 64 output rows. Total collective payload is ~25x
  smaller than the v1 AllGather-h1 scheme.
- Additive score pieces: layer-0 s_src+s_dst+mask and the layer-1 mask are
  folded into one host-precomputed per-block tile (ha0); layer-1 s_dst rides
  a PE outer-product broadcast, s_src rides the LeakyReLU activation bias.
"""

import sys

sys.path.insert(0, "/opt/trn_rl_repo")

import numpy as np
import ml_dtypes

from concourse import bass, bacc, mybir, tile
from concourse.bass_utils import run_bass_kernel_spmd

BF16 = ml_dtypes.bfloat16
FP8 = ml_dtypes.float8_e3m4

N = 512          # nodes
D = 768          # hidden
H = 12           # heads
F0 = 64          # layer-0 per-head dim
NCORES = 8
NPC = N // NCORES          # 64 local rows per core
NBLK = NPC // 4            # 16 blocks of 4 rows
ALPHA = 0.2
KT = D // 128              # 6 contraction tiles
W1H = 2 * D                # h1 columns per core (2 heads)
SPLIT = 12                 # e-blocks in the first AllToAll chunk
NEG = -9e15

E_FP8 = True               # stream e as fp8-e3m4 (else bf16)

F32 = mybir.dt.float32
BF = mybir.dt.bfloat16
EDT = mybir.dt.float8e3 if E_FP8 else mybir.dt.bfloat16
ENP = FP8 if E_FP8 else BF16
ADD = mybir.AluOpType.add
MULT = mybir.AluOpType.mult
AF = mybir.ActivationFunctionType

_COMPILED = None
DEBUG = False
SIM_SAFE = False  # replace Prelu (not in interp) with Relu for cost-model sims
_LAST_RESULTS = None


def _build_nc():
    nc = bacc.Bacc("TRN2", target_bir_lowering=False, num_devices=NCORES)
    lrelu_fn = AF.Relu if SIM_SAFE else AF.Prelu
    NA = 4 * SPLIT               # i-rows per src in AllToAll chunk a
    NB = NPC - NA                # i-rows per src in chunk b
    dbg = {}
    if DEBUG:
        dbg["sc0"] = nc.dram_tensor("dbg_sc0", [128, N], F32, kind="ExternalOutput")
        dbg["at0"] = nc.dram_tensor("dbg_at0", [128, N], F32, kind="ExternalOutput")
        dbg["at0T"] = nc.dram_tensor("dbg_at0T", [128, 128], F32, kind="ExternalOutput")
        dbg["x1p"] = nc.dram_tensor("dbg_x1p", [NPC, D], F32, kind="ExternalOutput")
        dbg["h0"] = nc.dram_tensor("dbg_h0", [128, D], F32, kind="ExternalOutput")
        dbg["x1"] = nc.dram_tensor("dbg_x1", [NPC, D], F32, kind="ExternalOutput")
        dbg["s1sel"] = nc.dram_tensor("dbg_s1sel", [4, N], F32, kind="ExternalOutput")
        dbg["sct"] = nc.dram_tensor("dbg_sct", [128, N], F32, kind="ExternalOutput")
        dbg["at1"] = nc.dram_tensor("dbg_at1", [128, N], F32, kind="ExternalOutput")
        dbg["h1q"] = nc.dram_tensor("dbg_h1q", [128, W1H], F32, kind="ExternalOutput")
        dbg["rsst"] = nc.dram_tensor("dbg_rsst", [128, D], F32, kind="ExternalOutput")

    eT_d = nc.dram_tensor("eT", [NBLK, 128, 24 * N], EDT, kind="ExternalInput")
    xT_d = nc.dram_tensor("xT", [128, KT * N], BF, kind="ExternalInput")
    w0r_d = nc.dram_tensor("w0r", [128, KT * D], BF, kind="ExternalInput")
    w1h_d = nc.dram_tensor("w1h", [128, KT * W1H], BF, kind="ExternalInput")
    v_d = nc.dram_tensor("vw", [128, KT * 32], BF, kind="ExternalInput")
    u1_d = nc.dram_tensor("u1sel", [128, KT * 4], BF, kind="ExternalInput")
    ha0_d = nc.dram_tensor("ha0", [NBLK, 128, N], BF, kind="ExternalInput")
    ident_d = nc.dram_tensor("ident", [128, 128], BF, kind="ExternalInput")
    # onesel[:, 128s:128(s+1)] is a [4,128] selector with row 2+s all-ones:
    # onesel[:, s].T @ s1s[0:4] broadcasts dst row s across 128 partitions.
    ones_d = nc.dram_tensor("onesel", [4, 256], BF, kind="ExternalInput")

    out_d = nc.dram_tensor("out", [NPC, D], F32, kind="ExternalOutput")

    at_in_a = nc.dram_tensor("at_in_a", [NCORES, NA, 2, N], BF)
    at_out_a = nc.dram_tensor("at_out_a", [NCORES, NA, 2, N], BF)
    at_in_b = nc.dram_tensor("at_in_b", [NCORES, NB, 2, N], BF)
    at_out_b = nc.dram_tensor("at_out_b", [NCORES, NB, 2, N], BF)
    agx_in = [
        nc.dram_tensor(f"agx_in{k}", [NPC // 2, D], BF) for k in range(2)
    ]
    agx_out = [
        nc.dram_tensor(f"agx_out{k}", [N // 2, D], BF, addr_space="Shared")
        for k in range(2)
    ]
    rs_in = nc.dram_tensor("rs_in", [N, D], F32)
    rs_out = nc.dram_tensor("rs_out", [NPC, D], F32)

    groups = [list(range(NCORES))]

    with tile.TileContext(nc) as tc:
        with (
            tc.tile_pool(name="const", bufs=1) as constp,
            tc.tile_pool(name="pers", bufs=1) as pers,
            tc.tile_pool(name="hapool", bufs=3) as hapool,
        ):
            # const loads go on the DVE/PE DMA queues so the sync queue can
            # start streaming e-tiles immediately
            ident = constp.tile([128, 128], BF, tag="ident")
            nc.gpsimd.dma_start(out=ident[:, :], in_=ident_d[:, :])
            ones = constp.tile([4, 256], BF, tag="ones")
            nc.gpsimd.dma_start(out=ones[:, :], in_=ones_d[:, :])
            w1h = constp.tile([128, KT * W1H], BF, tag="w1h")
            nc.gpsimd.dma_start(out=w1h[:, :], in_=w1h_d[:, :])
            u1 = constp.tile([128, KT * 4], BF, tag="u1")
            nc.gpsimd.dma_start(out=u1[:, :], in_=u1_d[:, :])

            x1T = pers.tile([128, KT * N], BF, tag="x1T")

            # =================== phase A: e-stream + layer 0 ===================
            with (
                tc.tile_pool(name="l0pers", bufs=1) as l0p,
                tc.tile_pool(name="l0const", bufs=1) as l0c,
                tc.tile_pool(name="epool", bufs=3) as epool,
                tc.tile_pool(name="l0work", bufs=2) as work,
            ):
                vw = l0c.tile([128, KT * 32], BF, tag="vw")
                nc.sync.dma_start(out=vw[:, :], in_=v_d[:, :])
                xT = l0c.tile([128, KT * N], BF, tag="xT")
                nc.scalar.dma_start(out=xT[:, :], in_=xT_d[:, :])
                w0r = l0c.tile([128, KT * D], BF, tag="w0r")
                nc.scalar.dma_start(out=w0r[:, :], in_=w0r_d[:, :])

                # ---- h0 = x @ W0r -> [4][128 nodes, 768] bf16 ----
                # (emitted inside the e-loop at b==2 so the first e-blocks'
                # matmuls keep the et-DMA pipeline primed)
                h0 = [l0p.tile([128, D], BF, tag=f"h0_{m}", name=f"h0_{m}") for m in range(4)]

                def h0_compute(psh0):
                    for m in range(4):
                        pa = psh0.tile([128, 512], F32, tag="ph0a")
                        pb = psh0.tile([128, 256], F32, tag="ph0b")
                        for k in range(KT):
                            lhs = xT[:, k * N + 128 * m : k * N + 128 * (m + 1)]
                            nc.tensor.matmul(
                                pa[:, :], lhs, w0r[:, k * D : k * D + 512],
                                start=(k == 0), stop=(k == KT - 1),
                            )
                            nc.tensor.matmul(
                                pb[:, :], lhs, w0r[:, k * D + 512 : (k + 1) * D],
                                start=(k == 0), stop=(k == KT - 1),
                            )
                        nc.vector.tensor_copy(out=h0[m][:, 0:512], in_=pa[:, :])
                        nc.vector.tensor_copy(out=h0[m][:, 512:768], in_=pb[:, :])
                    if DEBUG:
                        h0f = l0p.tile([128, D], F32, tag="h0f")
                        nc.vector.tensor_copy(out=h0f[:, :], in_=h0[0][:, :])
                        nc.sync.dma_start(out=dbg["h0"][:, :], in_=h0f[:, :])

                # ---- e-pass: scores, L1 staging, L0 softmax, att0^T ----
                at0T = [
                    l0p.tile([128, NBLK * 128], BF, tag=f"at0T_{q}", name=f"at0T_{q}") for q in range(4)
                ]
                with (
                    tc.tile_pool(name="psv", bufs=2, space="PSUM") as psvp,
                    tc.tile_pool(name="pst", bufs=2, space="PSUM") as pstp,
                    tc.tile_pool(name="psx1", bufs=1, space="PSUM") as psx1,
                    tc.tile_pool(name="psh0", bufs=1, space="PSUM") as psh0,
                ):

                    def x1_chunk(k):
                        """x1 rows [32k, 32k+32) = elu(att0 @ h0), then AllGather."""
                        px1a = psx1.tile([32, 512], F32, tag="px1a")
                        px1b = psx1.tile([32, 256], F32, tag="px1b")
                        for h in range(H):
                            dsti = (
                                px1a[:, 64 * h : 64 * (h + 1)]
                                if h < 8
                                else px1b[:, 64 * (h - 8) : 64 * (h - 7)]
                            )
                            for q in range(4):
                                lhs = at0T[q][:, :].rearrange(
                                    "p (b c r) -> p b c r", b=NBLK, c=4
                                )[:, 8 * k : 8 * k + 8, :, h : h + 1]
                                nc.tensor.matmul(
                                    dsti, lhs, h0[q][:, 64 * h : 64 * (h + 1)],
                                    start=(q == 0), stop=(q == 3),
                                )
                        x1p = work.tile([32, D], F32, tag="x1p")
                        nc.vector.tensor_copy(out=x1p[:, 0:512], in_=px1a[:, :])
                        nc.vector.tensor_copy(out=x1p[:, 512:768], in_=px1b[:, :])
                        tmin = work.tile([32, D], F32, tag="tmin")
                        nc.vector.tensor_scalar(
                            out=tmin[:, :], in0=x1p[:, :], scalar1=0.0, scalar2=None,
                            op0=mybir.AluOpType.min,
                        )
                        texp = work.tile([32, D], F32, tag="texp")
                        nc.scalar.activation(texp[:, :], tmin[:, :], AF.Exp)
                        tmax = work.tile([32, D], F32, tag="tmax")
                        nc.vector.tensor_scalar(
                            out=tmax[:, :], in0=x1p[:, :], scalar1=0.0, scalar2=None,
                            op0=mybir.AluOpType.max,
                        )
                        x1bf = work.tile([32, D], BF, tag="x1bf")
                        nc.vector.scalar_tensor_tensor(
                            out=x1bf[:, :], in0=texp[:, :], scalar=-1.0, in1=tmax[:, :],
                            op0=ADD, op1=ADD,
                        )
                        if DEBUG:
                            x1f32 = work.tile([32, D], F32, tag="x1f32")
                            nc.vector.scalar_tensor_tensor(
                                out=x1f32[:, :], in0=texp[:, :], scalar=-1.0,
                                in1=tmax[:, :], op0=ADD, op1=ADD,
                            )
                            nc.sync.dma_start(
                                out=dbg["x1"][32 * k : 32 * (k + 1), :], in_=x1f32[:, :]
                            )
                        nc.scalar.dma_start(out=agx_in[k][:, :], in_=x1bf[:, :])
                        nc.gpsimd.collective_compute(
                            "AllGather", mybir.AluOpType.bypass,
                            replica_groups=groups,
                            ins=[agx_in[k].ap().opt()], outs=[agx_out[k].ap().opt()],
                        )

                    for b in range(NBLK):
                        if b == 2:
                            h0_compute(psh0)
                        et = epool.tile([128, 24 * N], EDT, tag="etile")
                        nc.sync.dma_start(out=et[:, :], in_=eT_d[b])
                        ha = hapool.tile([128, N], BF, tag="ha0")
                        nc.scalar.dma_start(out=ha[:, :], in_=ha0_d[b])

                        psv = psvp.tile([128, N], F32, tag="psv")
                        for cc in range(4):
                            for kb in range(KT):
                                nc.tensor.matmul(
                                    psv[32 * cc : 32 * cc + 32, :],
                                    vw[:, 32 * kb : 32 * (kb + 1)],
                                    et[:, (cc * KT + kb) * N : (cc * KT + kb + 1) * N],
                                    start=(kb == 0), stop=(kb == KT - 1),
                                    tile_position=(0, 32 * cc),
                                )
                        # full scores (L0 rows 0:12, L1 rows 12:28 per cc-group)
                        sc0 = work.tile([128, N], BF, tag="sc0")
                        nc.vector.tensor_tensor(
                            out=sc0[:, :], in0=psv[:, :], in1=ha[:, :], op=ADD
                        )
                        if DEBUG and b == 0:
                            sc0f = work.tile([128, N], F32, tag="sc0f", bufs=1)
                            nc.vector.tensor_copy(out=sc0f[:, :], in_=sc0[:, :])
                            nc.sync.dma_start(out=dbg["sc0"][:, :], in_=sc0f[:, :])
                        # stage L1 rows to the AllToAll input buffer.
                        # NOTE: one DMA per cc-group — split-partition rearrange
                        # APs on SBUF tiles break tile dependency tracking.
                        for cc in range(4):
                            if b < SPLIT:
                                dst = at_in_a[:, 4 * b + cc, :, :]
                            else:
                                dst = at_in_b[:, 4 * (b - SPLIT) + cc, :, :]
                            # gpsimd only BEFORE the first collective
                            # (collectives block that queue for their whole
                            # duration); otherwise the Act queue.
                            if cc < 2:
                                eng = nc.scalar
                            elif b < 7:
                                eng = nc.gpsimd
                            else:
                                eng = nc.scalar
                            eng.dma_start(
                                out=dst,
                                in_=sc0[32 * cc + 12 : 32 * cc + 28, :],
                            )
                        # layer-0 softmax (valid rows cc*32+[0:12); rest harmless)
                        # LeakyReLU on DVE as max(x, 0.2x) keeps the Activation
                        # engine Exp-only: no act-table reloads.
                        tn0 = work.tile([128, N], BF, tag="tn0")
                        nc.vector.tensor_scalar(
                            out=tn0[:, :], in0=sc0[:, :], scalar1=ALPHA, scalar2=None,
                            op0=MULT,
                        )
                        lr0 = work.tile([128, N], BF, tag="lr0")
                        nc.vector.tensor_tensor(
                            out=lr0[:, :], in0=sc0[:, :], in1=tn0[:, :],
                            op=mybir.AluOpType.max,
                        )
                        ex0 = work.tile([128, N], BF, tag="ex0")
                        z0 = work.tile([128, 1], F32, tag="z0")
                        nc.scalar.activation(
                            ex0[:, :], lr0[:, :], AF.Exp, accum_out=z0[:, :]
                        )
                        rz0 = work.tile([128, 1], F32, tag="rz0")
                        nc.vector.reciprocal(rz0[:, :], z0[:, :])
                        at0 = work.tile([128, N], BF, tag="at0")
                        nc.vector.tensor_scalar(
                            out=at0[:, :], in0=ex0[:, :], scalar1=rz0[:, :],
                            scalar2=None, op0=MULT,
                        )
                        if DEBUG and b == 0:
                            at0f = work.tile([128, N], F32, tag="at0f", bufs=1)
                            nc.vector.tensor_copy(out=at0f[:, :], in_=at0[:, :])
                            nc.sync.dma_start(out=dbg["at0"][:, :], in_=at0f[:, :])
                        for q in range(4):
                            pt = pstp.tile([128, 128], BF, tag="ptr")
                            nc.tensor.transpose(
                                pt[:, :], at0[:, 128 * q : 128 * (q + 1)], ident[:, :]
                            )
                            nc.vector.tensor_copy(
                                out=at0T[q][:, 128 * b : 128 * (b + 1)], in_=pt[:, :]
                            )
                        if b == SPLIT - 1:
                            nc.gpsimd.collective_compute(
                                "AllToAll", mybir.AluOpType.bypass,
                                replica_groups=groups,
                                ins=[at_in_a.ap().opt()], outs=[at_out_a.ap().opt()],
                            )
                        if b == 7:
                            x1_chunk(0)
                    x1_chunk(1)

                # second AllToAll chunk (ready at stream end)
                nc.gpsimd.collective_compute(
                    "AllToAll", mybir.AluOpType.bypass,
                    replica_groups=groups,
                    ins=[at_in_b.ap().opt()], outs=[at_out_b.ap().opt()],
                )
                with tc.tile_pool(name="x1fp", bufs=1) as x1fp:
                    x1f = [x1fp.tile([128, D], BF, tag=f"x1f_{m}", name=f"x1f_{m}") for m in range(4)]
                    for m in range(4):
                        # global row j = 64*src + 32*k + r -> partition 64*ds+32*k+r
                        for k in range(2):
                            for ds in range(2):
                                nc.sync.dma_start(
                                    out=x1f[m][
                                        64 * ds + 32 * k : 64 * ds + 32 * k + 32, :
                                    ],
                                    in_=agx_out[k][
                                        32 * (2 * m + ds) : 32 * (2 * m + ds) + 32, :
                                    ],
                                )
                    with tc.tile_pool(name="psxt", bufs=2, space="PSUM") as psxt:
                        for m in range(4):
                            for k6 in range(KT):
                                pt = psxt.tile([128, 128], BF, tag="pxt")
                                nc.tensor.transpose(
                                    pt[:, :],
                                    x1f[m][:, 128 * k6 : 128 * (k6 + 1)],
                                    ident[:, :],
                                )
                                nc.vector.tensor_copy(
                                    out=x1T[
                                        :, N * k6 + 128 * m : N * k6 + 128 * (m + 1)
                                    ],
                                    in_=pt[:, :],
                                )

            # =================== tail: layer 1, head-sharded ===================
            with (
                tc.tile_pool(name="l1pers", bufs=1) as l1p,
                tc.tile_pool(name="l1work", bufs=2) as work,
                tc.tile_pool(name="scpool", bufs=3) as scpool,
                tc.tile_pool(name="at1pool", bufs=2) as at1pool,
            ):
                # s1sel = [src_A, src_B, dst_A, dst_B]^T [4, N]
                s1s = l1p.tile([4, N], BF, tag="s1s")
                srcT = l1p.tile([128, 8], BF, tag="srcT")  # [i, 2s] per g pair cols
                dm = [l1p.tile([128, N], BF, tag=f"dm_{s}", name=f"dm_{s}") for s in range(2)]
                with tc.tile_pool(name="pss1", bufs=1, space="PSUM") as pss1:
                    ps1 = pss1.tile([4, N], F32, tag="ps1")
                    for k in range(KT):
                        nc.tensor.matmul(
                            ps1[:, :], u1[:, 4 * k : 4 * (k + 1)],
                            x1T[:, N * k : N * (k + 1)],
                            start=(k == 0), stop=(k == KT - 1),
                        )
                    nc.vector.tensor_copy(out=s1s[:, :], in_=ps1[:, :])
                    if DEBUG:
                        s1f = work.tile([4, N], F32, tag="s1f", bufs=1)
                        nc.vector.tensor_copy(out=s1f[:, :], in_=ps1[:, :])
                        nc.sync.dma_start(out=dbg["s1sel"][:, :], in_=s1f[:, :])
                with tc.tile_pool(name="psdm", bufs=1, space="PSUM") as psdm:
                    # srcT[:, 2g+s] = s1sel[s, 128g:128(g+1)]
                    for g in range(4):
                        pt4 = psdm.tile([128, 2], BF, tag="pt4")
                        nc.tensor.transpose(
                            pt4[:, :], s1s[0:2, 128 * g : 128 * (g + 1)], ident[0:2, 0:2]
                        )
                        nc.vector.tensor_copy(
                            out=srcT[:, 2 * g : 2 * g + 2], in_=pt4[:, :]
                        )
                    # dm[s] = broadcast of dst row s over 128 partitions
                    for s in range(2):
                        pdm = psdm.tile([128, N], F32, tag="pdm")
                        nc.tensor.matmul(
                            pdm[:, :], ones[:, 128 * s : 128 * (s + 1)], s1s[0:4, :],
                            start=True, stop=True,
                        )
                        nc.vector.tensor_copy(out=dm[s][:, :], in_=pdm[:, :])

                # ---- h1 for my 2 heads: [4 jq][128, W1H] ----
                h1q = [l1p.tile([128, W1H], BF, tag=f"h1q_{q}", name=f"h1q_{q}") for q in range(4)]
                widths = [(0, 512), (512, 1024), (1024, 1536)]
                with tc.tile_pool(name="psh1", bufs=2, space="PSUM") as psh1:
                    for m in range(4):
                        ph1 = [
                            psh1.tile([128, 512], F32, tag="ph1a", name="ph1a"),
                            psh1.tile([128, 512], F32, tag="ph1b", name="ph1b"),
                            psh1.tile([128, 512], F32, tag="ph1c", name="ph1c"),
                        ]
                        for k in range(KT):
                            lhs = x1T[:, N * k + 128 * m : N * k + 128 * (m + 1)]
                            for t, (c0, c1) in enumerate(widths):
                                nc.tensor.matmul(
                                    ph1[t][:, 0 : c1 - c0], lhs,
                                    w1h[:, W1H * k + c0 : W1H * k + c1],
                                    start=(k == 0), stop=(k == KT - 1),
                                )
                        for t, (c0, c1) in enumerate(widths):
                            nc.scalar.copy(
                                out=h1q[m][:, c0:c1], in_=ph1[t][:, 0 : c1 - c0]
                            )

                # ---- per-igroup: softmax for both heads, att @ h1, partials ----
                with (
                    tc.tile_pool(name="pst1", bufs=2, space="PSUM") as pst1,
                    tc.tile_pool(name="pso", bufs=2, space="PSUM") as psop,
                ):
                    for g in range(4):
                        poa = psop.tile([128, 512], F32, tag="poa")
                        pob = psop.tile([128, 256], F32, tag="pob")
                        for s in range(2):
                            sct = scpool.tile([128, N], BF, tag="sct")
                            for ds in range(2):
                                nc.sync.dma_start(
                                    out=sct[64 * ds : 64 * ds + 4 * SPLIT, :],
                                    in_=at_out_a[2 * g + ds, :, s, :],
                                )
                                nc.sync.dma_start(
                                    out=sct[64 * ds + 4 * SPLIT : 64 * (ds + 1), :],
                                    in_=at_out_b[2 * g + ds, :, s, :],
                                )
                            sc1 = work.tile([128, N], BF, tag="sc1")
                            nc.vector.scalar_tensor_tensor(
                                out=sc1[:, :], in0=sct[:, :],
                                scalar=srcT[:, 2 * g + s : 2 * g + s + 1],
                                in1=dm[s][:, :], op0=ADD, op1=ADD,
                            )
                            if DEBUG and g == 0 and s == 0:
                                sctf = work.tile([128, N], F32, tag="sctf", bufs=1)
                                nc.vector.tensor_copy(out=sctf[:, :], in_=sc1[:, :])
                                nc.sync.dma_start(out=dbg["sct"][:, :], in_=sctf[:, :])
                            tn1 = work.tile([128, N], BF, tag="tn1")
                            nc.vector.tensor_scalar(
                                out=tn1[:, :], in0=sc1[:, :], scalar1=ALPHA,
                                scalar2=None, op0=MULT,
                            )
                            lr1 = work.tile([128, N], BF, tag="lr1")
                            nc.vector.tensor_tensor(
                                out=lr1[:, :], in0=sc1[:, :], in1=tn1[:, :],
                                op=mybir.AluOpType.max,
                            )
                            ex1 = work.tile([128, N], BF, tag="ex1")
                            z1 = work.tile([128, 1], F32, tag="z1")
                            nc.scalar.activation(
                                ex1[:, :], lr1[:, :], AF.Exp, accum_out=z1[:, :]
                            )
                            rz1 = work.tile([128, 1], F32, tag="rz1")
                            nc.vector.reciprocal(rz1[:, :], z1[:, :])
                            at1 = work.tile([128, N], BF, tag="at1")
                            if s == 1:
                                # head B is computed by two cores; halve it
                                nc.vector.tensor_scalar(
                                    out=at1[:, :], in0=ex1[:, :], scalar1=rz1[:, :],
                                    scalar2=0.5, op0=MULT, op1=MULT,
                                )
                            else:
                                nc.vector.tensor_scalar(
                                    out=at1[:, :], in0=ex1[:, :], scalar1=rz1[:, :],
                                    scalar2=None, op0=MULT,
                                )
                            if DEBUG and g == 0 and s == 0:
                                at1f = work.tile([128, N], F32, tag="at1f", bufs=1)
                                nc.vector.tensor_copy(out=at1f[:, :], in_=at1[:, :])
                                nc.sync.dma_start(out=dbg["at1"][:, :], in_=at1f[:, :])
                            at1T = at1pool.tile([128, 512], BF, tag="at1T")
                            for q in range(4):
                                pt = pst1.tile([128, 128], BF, tag="ptr1")
                                nc.tensor.transpose(
                                    pt[:, :], at1[:, 128 * q : 128 * (q + 1)], ident[:, :]
                                )
                                nc.vector.tensor_copy(
                                    out=at1T[:, 128 * q : 128 * (q + 1)], in_=pt[:, :]
                                )
                            for q in range(4):
                                lhsq = at1T[:, 128 * q : 128 * (q + 1)]
                                nc.tensor.matmul(
                                    poa[:, :], lhsq, h1q[q][:, D * s : D * s + 512],
                                    start=(s == 0 and q == 0), stop=(s == 1 and q == 3),
                                )
                                nc.tensor.matmul(
                                    pob[:, :], lhsq, h1q[q][:, D * s + 512 : D * (s + 1)],
                                    start=(s == 0 and q == 0), stop=(s == 1 and q == 3),
                                )
                        rsst = work.tile([128, D], F32, tag="rsst")
                        nc.vector.tensor_scalar(
                            out=rsst[:, 0:512], in0=poa[:, :], scalar1=1.0 / H,
                            scalar2=None, op0=MULT,
                        )
                        nc.vector.tensor_scalar(
                            out=rsst[:, 512:768], in0=pob[:, :], scalar1=1.0 / H,
                            scalar2=None, op0=MULT,
                        )
                        if DEBUG and g == 0:
                            nc.sync.dma_start(out=dbg["rsst"][:, :], in_=rsst[:, :])
                        nc.sync.dma_start(
                            out=rs_in[128 * g : 128 * (g + 1), :], in_=rsst[:, :]
                        )

                # ---- ReduceScatter partial outputs -> my 64 rows ----
                nc.gpsimd.collective_compute(
                    "ReduceScatter", ADD,
                    replica_groups=groups,
                    ins=[rs_in.ap().opt()], outs=[rs_out.ap().opt()],
                )
                opf = work.tile([64, D], F32, tag="opf", bufs=1)
                nc.sync.dma_start(out=opf[:, :], in_=rs_out[:, :])
                omin = work.tile([64, D], F32, tag="omin", bufs=1)
                nc.vector.tensor_scalar(
                    out=omin[:, :], in0=opf[:, :], scalar1=0.0, scalar2=None,
                    op0=mybir.AluOpType.min,
                )
                oexp = work.tile([64, D], F32, tag="oexp", bufs=1)
                nc.scalar.activation(oexp[:, :], omin[:, :], AF.Exp)
                omax = work.tile([64, D], F32, tag="omax", bufs=1)
                nc.vector.tensor_scalar(
                    out=omax[:, :], in0=opf[:, :], scalar1=0.0, scalar2=None,
                    op0=mybir.AluOpType.max,
                )
                ofin = work.tile([64, D], F32, tag="ofin", bufs=1)
                nc.vector.scalar_tensor_tensor(
                    out=ofin[:, :], in0=oexp[:, :], scalar=-1.0, in1=omax[:, :],
                    op0=ADD, op1=ADD,
                )
                nc.scalar.dma_start(out=out_d[:, :], in_=ofin[:, :])

    nc.compile()
    return nc


def _fold_weights(We, W, a, F_):
    We = We.astype(np.float64)
    W = W.astype(np.float64)
    a = a.astype(np.float64)
    a1, a2, a3 = a[:, :F_], a[:, F_ : 2 * F_], a[:, 2 * F_ :]
    v = np.einsum("hei,hif,hf->he", We, W, a3)
    usrc = np.einsum("hif,hf->hi", W, a1)
    udst = np.einsum("hif,hf->hi", W, a2)
    return v, usrc, udst


def _to_ktile(mat):
    """[768, C] -> [128, KT*C] with the KT k-tiles side by side."""
    k, c = mat.shape
    assert k == D
    return np.ascontiguousarray(
        mat.reshape(KT, 128, c).transpose(1, 0, 2).reshape(128, KT * c)
    )


def kernel(**inputs):
    global _COMPILED
    x = np.asarray(inputs["x"], dtype=np.float32)
    adj = np.asarray(inputs["adj"])
    e = np.asarray(inputs["e"], dtype=np.float32)
    W0 = np.asarray(inputs["W0"], dtype=np.float32)
    a0 = np.asarray(inputs["a0"], dtype=np.float32)
    W1 = np.asarray(inputs["W1"], dtype=np.float32)
    a1_ = np.asarray(inputs["a1"], dtype=np.float32)
    We0 = np.asarray(inputs["We0"], dtype=np.float32)
    We1 = np.asarray(inputs["We1"], dtype=np.float32)

    v0, _, _ = _fold_weights(We0, W0, a0, F0)
    v1, u1src, u1dst = _fold_weights(We1, W1, a1_, D)

    # V slot layout: 0-11 = layer-0 heads; 12+2g+s: s=0 -> head g, s=1 -> head 8+g//2
    V32 = np.zeros((D, 32), np.float64)
    V32[:, :12] = v0.T
    for g in range(NCORES):
        V32[:, 12 + 2 * g] = v1[g]
        V32[:, 12 + 2 * g + 1] = v1[8 + g // 2]
    v_bf = _to_ktile(V32.astype(np.float32)).astype(BF16)

    h0h = np.einsum("ni,hif->hnf", x.astype(np.float64), W0.astype(np.float64))
    s_src0 = np.einsum("hnf,hf->hn", h0h, a0[:, :F0].astype(np.float64))
    s_dst0 = np.einsum("hnf,hf->hn", h0h, a0[:, F0 : 2 * F0].astype(np.float64))
    maskadd = (adj.astype(np.float32) - 1.0) * 9e15                   # 0 or -9e15

    xT_bf = _to_ktile(np.ascontiguousarray(x.T)).astype(BF16)
    w0r_bf = _to_ktile(W0.transpose(1, 0, 2).reshape(D, H * F0)).astype(BF16)
    W1r = W1.transpose(1, 0, 2).reshape(D, H * D)
    ident = np.eye(128, dtype=np.float32).astype(BF16)
    onesel = np.zeros((4, 256), np.float32)
    onesel[2, 0:128] = 1.0
    onesel[3, 128:256] = 1.0
    onesel = onesel.astype(BF16)

    # block-major fp8 e layout: eb[c, b, p, cc, kb, j] = e[64c+4b+cc, j, 128kb+p]
    e8 = e.astype(ENP)                                   # [i, j, k]
    v8 = e8.reshape(NCORES, NBLK, 4, N, KT, 128)          # [c, b, cc, j, kb, p]
    eb = np.ascontiguousarray(v8.transpose(0, 1, 5, 2, 4, 3)).reshape(
        NCORES, NBLK, 128, 24 * N
    )

    in_maps = []
    for c in range(NCORES):
        hA = c
        hB = 8 + c // 2
        ha0 = np.zeros((NBLK, 128, N), dtype=np.float32)
        for b in range(NBLK):
            for cc in range(4):
                i = NPC * c + 4 * b + cc
                ha0[b, 32 * cc : 32 * cc + 12, :] = (
                    s_dst0 + s_src0[:, i : i + 1] + maskadd[i : i + 1, :]
                )
                ha0[b, 32 * cc + 12 : 32 * cc + 28, :] = maskadd[i : i + 1, :]
        w1h_bf = _to_ktile(
            np.ascontiguousarray(
                np.concatenate(
                    [W1r[:, D * hA : D * (hA + 1)], W1r[:, D * hB : D * (hB + 1)]],
                    axis=1,
                )
            )
        ).astype(BF16)
        u1sel = np.stack(
            [u1src[hA], u1src[hB], u1dst[hA], u1dst[hB]], axis=1
        ).astype(np.float32)                                # [768, 4]
        in_maps.append(
            {
                "eT": eb[c],
                "xT": xT_bf,
                "w0r": w0r_bf,
                "w1h": w1h_bf,
                "vw": v_bf,
                "u1sel": _to_ktile(u1sel).astype(BF16),
                "ha0": ha0.astype(BF16),
                "ident": ident,
                "onesel": onesel,
            }
        )

    if _COMPILED is None:
        _COMPILED = _build_nc()
    nc = _COMPILED

    res = run_bass_kernel_spmd(nc, in_maps, list(range(NCORES)))
    global _LAST_RESULTS
    _LAST_RESULTS = res.results
    out = np.concatenate([res.results[c]["out"] for c in range(NCORES)], axis=0)
    return out.astype(np.float32)


if __name__ == "__main__":
    import reference

    inputs = {k: np.asarray(v) for k, v in reference.setup_inputs().items()}
    got = kernel(**inputs)
    print("output shape:", got.shape, got.dtype)


# revision 46
# speedup vs baseline: 1.0113x; 1.0113x over previous
"""Bass/Trainium2 kernel for a 2-layer multi-head GAT (DocRE model).

Contract: kernel(**inputs) takes the FULL unsharded inputs as numpy arrays
and returns the FULL [512, 768] float32 output.

Sharding / dataflow (v2, collective-light):
- Layer 0 is row-sharded: each core streams its 64 rows of the edge tensor e
  once (block-major fp8-e3m4 layout) and computes BOTH layers' edge scores
  s_e = e @ V in a single pass (V holds 12 layer-0 head columns plus 16
  layer-1 "slot" columns = (dst core, slot) pairs).
- Layer-1 edge scores are AllToAll'd to a head-sharded layout DURING the
  e-stream (2 chunks; the first is hidden under the stream). Each core then
  owns 2 layer-1 heads: head A = core id (weight 1), head B = 8 + core//2
  (computed by two cores, each weighted 1/2).
- x1 is AllGathered once (small); h1 columns for the core's two heads are
  computed locally from the gathered x1 -> NO AllGather of the big h1.
- Per-head partial outputs (all 512 rows) are ReduceScatter'd (fp32 add) to
  give each core its 64 output rows. Total collective payload is ~25x
  smaller than the v1 AllGather-h1 scheme.
- Additive score pieces: layer-0 s_src+s_dst+mask and the layer-1 mask are
  folded into one host-precomputed per-block tile (ha0); layer-1 s_dst rides
  a PE outer-product broadcast, s_src rides the LeakyReLU activation bias.
"""

import sys

sys.path.insert(0, "/opt/trn_rl_repo")

import numpy as np
import ml_dtypes

from concourse import bass, bacc, mybir, tile
from concourse.bass_utils import run_bass_kernel_spmd

BF16 = ml_dtypes.bfloat16
FP8 = ml_dtypes.float8_e3m4

N = 512          # nodes
D = 768          # hidden
H = 12           # heads
F0 = 64          # layer-0 per-head dim
NCORES = 8
NPC = N // NCORES          # 64 local rows per core
NBLK = NPC // 4            # 16 blocks of 4 rows
ALPHA = 0.2
KT = D // 128              # 6 contraction tiles
W1H = 2 * D                # h1 columns per core (2 heads)
SPLIT = 12                 # e-blocks in the first AllToAll chunk
NEG = -9e15

E_FP8 = True               # stream e as fp8-e3m4 (else bf16)

F32 = mybir.dt.float32
BF = mybir.dt.bfloat16
EDT = mybir.dt.float8e3 if E_FP8 else mybir.dt.bfloat16
ENP = FP8 if E_FP8 else BF16
ADD = mybir.AluOpType.add
MULT = mybir.AluOpType.mult
AF = mybir.ActivationFunctionType

_COMPILED = None
DEBUG = False
SIM_SAFE = False  # replace Prelu (not in interp) with Relu for cost-model sims
_LAST_RESULTS = None


def _build_nc():
    nc = bacc.Bacc("TRN2", target_bir_lowering=False, num_devices=NCORES)
    lrelu_fn = AF.Relu if SIM_SAFE else AF.Prelu
    NA = 4 * SPLIT               # i-rows per src in AllToAll chunk a
    NB = NPC - NA                # i-rows per src in chunk b
    dbg = {}
    if DEBUG:
        dbg["sc0"] = nc.dram_tensor("dbg_sc0", [128, N], F32, kind="ExternalOutput")
        dbg["at0"] = nc.dram_tensor("dbg_at0", [128, N], F32, kind="ExternalOutput")
        dbg["at0T"] = nc.dram_tensor("dbg_at0T", [128, 128], F32, kind="ExternalOutput")
        dbg["x1p"] = nc.dram_tensor("dbg_x1p", [NPC, D], F32, kind="ExternalOutput")
        dbg["h0"] = nc.dram_tensor("dbg_h0", [128, D], F32, kind="ExternalOutput")
        dbg["x1"] = nc.dram_tensor("dbg_x1", [NPC, D], F32, kind="ExternalOutput")
        dbg["s1sel"] = nc.dram_tensor("dbg_s1sel", [4, N], F32, kind="ExternalOutput")
        dbg["sct"] = nc.dram_tensor("dbg_sct", [128, N], F32, kind="ExternalOutput")
        dbg["at1"] = nc.dram_tensor("dbg_at1", [128, N], F32, kind="ExternalOutput")
        dbg["h1q"] = nc.dram_tensor("dbg_h1q", [128, W1H], F32, kind="ExternalOutput")
        dbg["rsst"] = nc.dram_tensor("dbg_rsst", [128, D], F32, kind="ExternalOutput")

    eT_d = nc.dram_tensor("eT", [NBLK, 128, 24 * N], EDT, kind="ExternalInput")
    xT_d = nc.dram_tensor("xT", [128, KT * N], BF, kind="ExternalInput")
    w0r_d = nc.dram_tensor("w0r", [128, KT * D], BF, kind="ExternalInput")
    w1h_d = nc.dram_tensor("w1h", [128, KT * W1H], BF, kind="ExternalInput")
    v_d = nc.dram_tensor("vw", [128, KT * 32], BF, kind="ExternalInput")
    u1_d = nc.dram_tensor("u1sel", [128, KT * 4], BF, kind="ExternalInput")
    ha0_d = nc.dram_tensor("ha0", [NBLK, 128, N], BF, kind="ExternalInput")
    ident_d = nc.dram_tensor("ident", [128, 128], BF, kind="ExternalInput")
    # onesel[:, 128s:128(s+1)] is a [4,128] selector with row 2+s all-ones:
    # onesel[:, s].T @ s1s[0:4] broadcasts dst row s across 128 partitions.
    ones_d = nc.dram_tensor("onesel", [4, 256], BF, kind="ExternalInput")

    out_d = nc.dram_tensor("out", [NPC, D], F32, kind="ExternalOutput")

    at_in_a = nc.dram_tensor("at_in_a", [NCORES, NA, 2, N], BF)
    at_out_a = nc.dram_tensor("at_out_a", [NCORES, NA, 2, N], BF)
    agx_in = [
        nc.dram_tensor(f"agx_in{k}", [NPC // 2, D], BF) for k in range(2)
    ]
    agx_out = [
        nc.dram_tensor(f"agx_out{k}", [N // 2, D], BF, addr_space="Shared")
        for k in range(2)
    ]
    # chunk-b AllToAll payload per dst: [NB, 2, N] scores + [32, D] x1 chunk-1
    # (replicated to every dst -> the AllToAll doubles as the 2nd x1 AllGather)
    RB = NB * 2 * N + 32 * D
    at_in_b = nc.dram_tensor("at_in_b", [NCORES, RB], BF)
    at_out_b = nc.dram_tensor("at_out_b", [NCORES, RB], BF)
    rs_in = nc.dram_tensor("rs_in", [N, D], F32)
    rs_out = nc.dram_tensor("rs_out", [NPC, D], F32)

    groups = [list(range(NCORES))]

    with tile.TileContext(nc) as tc:
        with (
            tc.tile_pool(name="const", bufs=1) as constp,
            tc.tile_pool(name="pers", bufs=1) as pers,
            tc.tile_pool(name="hapool", bufs=3) as hapool,
        ):
            # const loads go on the DVE/PE DMA queues so the sync queue can
            # start streaming e-tiles immediately
            ident = constp.tile([128, 128], BF, tag="ident")
            nc.gpsimd.dma_start(out=ident[:, :], in_=ident_d[:, :])
            ones = constp.tile([4, 256], BF, tag="ones")
            nc.gpsimd.dma_start(out=ones[:, :], in_=ones_d[:, :])
            w1h = constp.tile([128, KT * W1H], BF, tag="w1h")
            nc.gpsimd.dma_start(out=w1h[:, :], in_=w1h_d[:, :])
            u1 = constp.tile([128, KT * 4], BF, tag="u1")
            nc.gpsimd.dma_start(out=u1[:, :], in_=u1_d[:, :])

            x1T = pers.tile([128, KT * N], BF, tag="x1T")

            # =================== phase A: e-stream + layer 0 ===================
            with (
                tc.tile_pool(name="l0pers", bufs=1) as l0p,
                tc.tile_pool(name="l0const", bufs=1) as l0c,
                tc.tile_pool(name="epool", bufs=3) as epool,
                tc.tile_pool(name="l0work", bufs=2) as work,
            ):
                vw = l0c.tile([128, KT * 32], BF, tag="vw")
                nc.sync.dma_start(out=vw[:, :], in_=v_d[:, :])
                xT = l0c.tile([128, KT * N], BF, tag="xT")
                nc.scalar.dma_start(out=xT[:, :], in_=xT_d[:, :])
                w0r = l0c.tile([128, KT * D], BF, tag="w0r")
                nc.scalar.dma_start(out=w0r[:, :], in_=w0r_d[:, :])

                # ---- h0 = x @ W0r -> [4][128 nodes, 768] bf16 ----
                # (emitted inside the e-loop at b==2 so the first e-blocks'
                # matmuls keep the et-DMA pipeline primed)
                h0 = [l0p.tile([128, D], BF, tag=f"h0_{m}", name=f"h0_{m}") for m in range(4)]

                def h0_compute(psh0):
                    for m in range(4):
                        pa = psh0.tile([128, 512], F32, tag="ph0a")
                        pb = psh0.tile([128, 256], F32, tag="ph0b")
                        for k in range(KT):
                            lhs = xT[:, k * N + 128 * m : k * N + 128 * (m + 1)]
                            nc.tensor.matmul(
                                pa[:, :], lhs, w0r[:, k * D : k * D + 512],
                                start=(k == 0), stop=(k == KT - 1),
                            )
                            nc.tensor.matmul(
                                pb[:, :], lhs, w0r[:, k * D + 512 : (k + 1) * D],
                                start=(k == 0), stop=(k == KT - 1),
                            )
                        nc.vector.tensor_copy(out=h0[m][:, 0:512], in_=pa[:, :])
                        nc.vector.tensor_copy(out=h0[m][:, 512:768], in_=pb[:, :])
                    if DEBUG:
                        h0f = l0p.tile([128, D], F32, tag="h0f")
                        nc.vector.tensor_copy(out=h0f[:, :], in_=h0[0][:, :])
                        nc.sync.dma_start(out=dbg["h0"][:, :], in_=h0f[:, :])

                # ---- e-pass: scores, L1 staging, L0 softmax, att0^T ----
                at0T = [
                    l0p.tile([128, NBLK * 128], BF, tag=f"at0T_{q}", name=f"at0T_{q}") for q in range(4)
                ]
                with (
                    tc.tile_pool(name="psv", bufs=2, space="PSUM") as psvp,
                    tc.tile_pool(name="pst", bufs=2, space="PSUM") as pstp,
                    tc.tile_pool(name="psx1", bufs=1, space="PSUM") as psx1,
                    tc.tile_pool(name="psh0", bufs=1, space="PSUM") as psh0,
                ):

                    def x1_chunk(k):
                        """x1 rows [32k, 32k+32) = elu(att0 @ h0), then AllGather."""
                        px1a = psx1.tile([32, 512], F32, tag="px1a")
                        px1b = psx1.tile([32, 256], F32, tag="px1b")
                        for h in range(H):
                            dsti = (
                                px1a[:, 64 * h : 64 * (h + 1)]
                                if h < 8
                                else px1b[:, 64 * (h - 8) : 64 * (h - 7)]
                            )
                            for q in range(4):
                                lhs = at0T[q][:, :].rearrange(
                                    "p (b c r) -> p b c r", b=NBLK, c=4
                                )[:, 8 * k : 8 * k + 8, :, h : h + 1]
                                nc.tensor.matmul(
                                    dsti, lhs, h0[q][:, 64 * h : 64 * (h + 1)],
                                    start=(q == 0), stop=(q == 3),
                                )
                        x1p = work.tile([32, D], F32, tag="x1p")
                        nc.vector.tensor_copy(out=x1p[:, 0:512], in_=px1a[:, :])
                        nc.vector.tensor_copy(out=x1p[:, 512:768], in_=px1b[:, :])
                        tmin = work.tile([32, D], F32, tag="tmin")
                        nc.vector.tensor_scalar(
                            out=tmin[:, :], in0=x1p[:, :], scalar1=0.0, scalar2=None,
                            op0=mybir.AluOpType.min,
                        )
                        texp = work.tile([32, D], F32, tag="texp")
                        nc.scalar.activation(texp[:, :], tmin[:, :], AF.Exp)
                        tmax = work.tile([32, D], F32, tag="tmax")
                        nc.vector.tensor_scalar(
                            out=tmax[:, :], in0=x1p[:, :], scalar1=0.0, scalar2=None,
                            op0=mybir.AluOpType.max,
                        )
                        x1bf = work.tile([32, D], BF, tag="x1bf")
                        nc.vector.scalar_tensor_tensor(
                            out=x1bf[:, :], in0=texp[:, :], scalar=-1.0, in1=tmax[:, :],
                            op0=ADD, op1=ADD,
                        )
                        if DEBUG:
                            x1f32 = work.tile([32, D], F32, tag="x1f32")
                            nc.vector.scalar_tensor_tensor(
                                out=x1f32[:, :], in0=texp[:, :], scalar=-1.0,
                                in1=tmax[:, :], op0=ADD, op1=ADD,
                            )
                            nc.sync.dma_start(
                                out=dbg["x1"][32 * k : 32 * (k + 1), :], in_=x1f32[:, :]
                            )
                        if k == 0:
                            nc.scalar.dma_start(out=agx_in[k][:, :], in_=x1bf[:, :])
                            nc.gpsimd.collective_compute(
                                "AllGather", mybir.AluOpType.bypass,
                                replica_groups=groups,
                                ins=[agx_in[k].ap().opt()], outs=[agx_out[k].ap().opt()],
                            )
                        else:
                            # replicate x1 chunk-1 into every dst's AllToAll
                            # block: AT_b doubles as the 2nd x1 AllGather.
                            for g in range(NCORES):
                                eng = nc.scalar if g % 2 == 0 else nc.sync
                                eng.dma_start(
                                    out=at_in_b[
                                        g, NB * 2 * N : NB * 2 * N + 32 * D
                                    ].rearrange("(r f) -> r f", r=32),
                                    in_=x1bf[:, :],
                                )

                    for b in range(NBLK):
                        if b == 2:
                            h0_compute(psh0)
                        et = epool.tile([128, 24 * N], EDT, tag="etile")
                        nc.sync.dma_start(out=et[:, :], in_=eT_d[b])
                        ha = hapool.tile([128, N], BF, tag="ha0")
                        nc.scalar.dma_start(out=ha[:, :], in_=ha0_d[b])

                        psv = psvp.tile([128, N], F32, tag="psv")
                        for cc in range(4):
                            for kb in range(KT):
                                nc.tensor.matmul(
                                    psv[32 * cc : 32 * cc + 32, :],
                                    vw[:, 32 * kb : 32 * (kb + 1)],
                                    et[:, (cc * KT + kb) * N : (cc * KT + kb + 1) * N],
                                    start=(kb == 0), stop=(kb == KT - 1),
                                    tile_position=(0, 32 * cc),
                                )
                        # full scores (L0 rows 0:12, L1 rows 12:28 per cc-group)
                        sc0 = work.tile([128, N], BF, tag="sc0")
                        nc.vector.tensor_tensor(
                            out=sc0[:, :], in0=psv[:, :], in1=ha[:, :], op=ADD
                        )
                        if DEBUG and b == 0:
                            sc0f = work.tile([128, N], F32, tag="sc0f", bufs=1)
                            nc.vector.tensor_copy(out=sc0f[:, :], in_=sc0[:, :])
                            nc.sync.dma_start(out=dbg["sc0"][:, :], in_=sc0f[:, :])
                        # stage L1 rows to the AllToAll input buffer.
                        # NOTE: one DMA per cc-group — split-partition rearrange
                        # APs on SBUF tiles break tile dependency tracking.
                        for cc in range(4):
                            if b < SPLIT:
                                dst = at_in_a[:, 4 * b + cc, :, :]
                            else:
                                off = (4 * (b - SPLIT) + cc) * 2 * N
                                dst = at_in_b[:, off : off + 2 * N].rearrange(
                                    "g (s j) -> g s j", s=2, j=N
                                )
                            # gpsimd only BEFORE the first collective
                            # (collectives block that queue for their whole
                            # duration); otherwise the Act queue.
                            if cc < 2:
                                eng = nc.scalar
                            elif b < 7:
                                eng = nc.gpsimd
                            else:
                                eng = nc.scalar
                            eng.dma_start(
                                out=dst,
                                in_=sc0[32 * cc + 12 : 32 * cc + 28, :],
                            )
                        # layer-0 softmax (valid rows cc*32+[0:12); rest harmless)
                        # LeakyReLU on DVE as max(x, 0.2x) keeps the Activation
                        # engine Exp-only: no act-table reloads.
                        tn0 = work.tile([128, N], BF, tag="tn0")
                        nc.vector.tensor_scalar(
                            out=tn0[:, :], in0=sc0[:, :], scalar1=ALPHA, scalar2=None,
                            op0=MULT,
                        )
                        lr0 = work.tile([128, N], BF, tag="lr0")
                        nc.vector.tensor_tensor(
                            out=lr0[:, :], in0=sc0[:, :], in1=tn0[:, :],
                            op=mybir.AluOpType.max,
                        )
                        ex0 = work.tile([128, N], BF, tag="ex0")
                        z0 = work.tile([128, 1], F32, tag="z0")
                        nc.scalar.activation(
                            ex0[:, :], lr0[:, :], AF.Exp, accum_out=z0[:, :]
                        )
                        rz0 = work.tile([128, 1], F32, tag="rz0")
                        nc.vector.reciprocal(rz0[:, :], z0[:, :])
                        at0 = work.tile([128, N], BF, tag="at0")
                        nc.vector.tensor_scalar(
                            out=at0[:, :], in0=ex0[:, :], scalar1=rz0[:, :],
                            scalar2=None, op0=MULT,
                        )
                        if DEBUG and b == 0:
                            at0f = work.tile([128, N], F32, tag="at0f", bufs=1)
                            nc.vector.tensor_copy(out=at0f[:, :], in_=at0[:, :])
                            nc.sync.dma_start(out=dbg["at0"][:, :], in_=at0f[:, :])
                        for q in range(4):
                            pt = pstp.tile([128, 128], BF, tag="ptr")
                            nc.tensor.transpose(
                                pt[:, :], at0[:, 128 * q : 128 * (q + 1)], ident[:, :]
                            )
                            nc.vector.tensor_copy(
                                out=at0T[q][:, 128 * b : 128 * (b + 1)], in_=pt[:, :]
                            )
                        if b == SPLIT - 1:
                            nc.gpsimd.collective_compute(
                                "AllToAll", mybir.AluOpType.bypass,
                                replica_groups=groups,
                                ins=[at_in_a.ap().opt()], outs=[at_out_a.ap().opt()],
                            )
                        if b == 7:
                            x1_chunk(0)
                    x1_chunk(1)

                # second AllToAll chunk (ready at stream end)
                nc.gpsimd.collective_compute(
                    "AllToAll", mybir.AluOpType.bypass,
                    replica_groups=groups,
                    ins=[at_in_b.ap().opt()], outs=[at_out_b.ap().opt()],
                )
                with tc.tile_pool(name="x1fp", bufs=1) as x1fp:
                    x1f = [x1fp.tile([128, D], BF, tag=f"x1f_{m}", name=f"x1f_{m}") for m in range(4)]
                    for m in range(4):
                        # global row j = 64*src + 32*k + r -> partition 64*ds+32*k+r
                        for ds in range(2):
                            nc.sync.dma_start(
                                out=x1f[m][64 * ds : 64 * ds + 32, :],
                                in_=agx_out[0][
                                    32 * (2 * m + ds) : 32 * (2 * m + ds) + 32, :
                                ],
                            )
                            nc.sync.dma_start(
                                out=x1f[m][64 * ds + 32 : 64 * ds + 64, :],
                                in_=at_out_b[
                                    2 * m + ds, NB * 2 * N : NB * 2 * N + 32 * D
                                ].rearrange("(r f) -> r f", r=32),
                            )
                    with tc.tile_pool(name="psxt", bufs=2, space="PSUM") as psxt:
                        for m in range(4):
                            for k6 in range(KT):
                                pt = psxt.tile([128, 128], BF, tag="pxt")
                                nc.tensor.transpose(
                                    pt[:, :],
                                    x1f[m][:, 128 * k6 : 128 * (k6 + 1)],
                                    ident[:, :],
                                )
                                nc.vector.tensor_copy(
                                    out=x1T[
                                        :, N * k6 + 128 * m : N * k6 + 128 * (m + 1)
                                    ],
                                    in_=pt[:, :],
                                )

            # =================== tail: layer 1, head-sharded ===================
            with (
                tc.tile_pool(name="l1pers", bufs=1) as l1p,
                tc.tile_pool(name="l1work", bufs=2) as work,
                tc.tile_pool(name="scpool", bufs=3) as scpool,
                tc.tile_pool(name="at1pool", bufs=2) as at1pool,
            ):
                # s1sel = [src_A, src_B, dst_A, dst_B]^T [4, N]
                s1s = l1p.tile([4, N], BF, tag="s1s")
                srcT = l1p.tile([128, 8], BF, tag="srcT")  # [i, 2s] per g pair cols
                dm = [l1p.tile([128, N], BF, tag=f"dm_{s}", name=f"dm_{s}") for s in range(2)]
                with tc.tile_pool(name="pss1", bufs=1, space="PSUM") as pss1:
                    ps1 = pss1.tile([4, N], F32, tag="ps1")
                    for k in range(KT):
                        nc.tensor.matmul(
                            ps1[:, :], u1[:, 4 * k : 4 * (k + 1)],
                            x1T[:, N * k : N * (k + 1)],
                            start=(k == 0), stop=(k == KT - 1),
                        )
                    nc.vector.tensor_copy(out=s1s[:, :], in_=ps1[:, :])
                    if DEBUG:
                        s1f = work.tile([4, N], F32, tag="s1f", bufs=1)
                        nc.vector.tensor_copy(out=s1f[:, :], in_=ps1[:, :])
                        nc.sync.dma_start(out=dbg["s1sel"][:, :], in_=s1f[:, :])
                with tc.tile_pool(name="psdm", bufs=1, space="PSUM") as psdm:
                    # srcT[:, 2g+s] = s1sel[s, 128g:128(g+1)]
                    for g in range(4):
                        pt4 = psdm.tile([128, 2], BF, tag="pt4")
                        nc.tensor.transpose(
                            pt4[:, :], s1s[0:2, 128 * g : 128 * (g + 1)], ident[0:2, 0:2]
                        )
                        nc.vector.tensor_copy(
                            out=srcT[:, 2 * g : 2 * g + 2], in_=pt4[:, :]
                        )
                    # dm[s] = broadcast of dst row s over 128 partitions
                    for s in range(2):
                        pdm = psdm.tile([128, N], F32, tag="pdm")
                        nc.tensor.matmul(
                            pdm[:, :], ones[:, 128 * s : 128 * (s + 1)], s1s[0:4, :],
                            start=True, stop=True,
                        )
                        nc.vector.tensor_copy(out=dm[s][:, :], in_=pdm[:, :])

                # ---- h1 for my 2 heads: [4 jq][128, W1H] ----
                h1q = [l1p.tile([128, W1H], BF, tag=f"h1q_{q}", name=f"h1q_{q}") for q in range(4)]
                widths = [(0, 512), (512, 1024), (1024, 1536)]
                with tc.tile_pool(name="psh1", bufs=2, space="PSUM") as psh1:
                    for m in range(4):
                        ph1 = [
                            psh1.tile([128, 512], F32, tag="ph1a", name="ph1a"),
                            psh1.tile([128, 512], F32, tag="ph1b", name="ph1b"),
                            psh1.tile([128, 512], F32, tag="ph1c", name="ph1c"),
                        ]
                        for k in range(KT):
                            lhs = x1T[:, N * k + 128 * m : N * k + 128 * (m + 1)]
                            for t, (c0, c1) in enumerate(widths):
                                nc.tensor.matmul(
                                    ph1[t][:, 0 : c1 - c0], lhs,
                                    w1h[:, W1H * k + c0 : W1H * k + c1],
                                    start=(k == 0), stop=(k == KT - 1),
                                )
                        for t, (c0, c1) in enumerate(widths):
                            nc.scalar.copy(
                                out=h1q[m][:, c0:c1], in_=ph1[t][:, 0 : c1 - c0]
                            )

                # ---- per-igroup: softmax for both heads, att @ h1, partials ----
                with (
                    tc.tile_pool(name="pst1", bufs=2, space="PSUM") as pst1,
                    tc.tile_pool(name="pso", bufs=2, space="PSUM") as psop,
                ):
                    for g in range(4):
                        poa = psop.tile([128, 512], F32, tag="poa")
                        pob = psop.tile([128, 256], F32, tag="pob")
                        for s in range(2):
                            sct = scpool.tile([128, N], BF, tag="sct")
                            for ds in range(2):
                                nc.sync.dma_start(
                                    out=sct[64 * ds : 64 * ds + 4 * SPLIT, :],
                                    in_=at_out_a[2 * g + ds, :, s, :],
                                )
                                nc.sync.dma_start(
                                    out=sct[64 * ds + 4 * SPLIT : 64 * (ds + 1), :],
                                    in_=at_out_b[
                                        2 * g + ds, 0 : NB * 2 * N
                                    ].rearrange("(i s j) -> i s j", s=2, j=N)[:, s, :],
                                )
                            sc1 = work.tile([128, N], BF, tag="sc1")
                            nc.vector.scalar_tensor_tensor(
                                out=sc1[:, :], in0=sct[:, :],
                                scalar=srcT[:, 2 * g + s : 2 * g + s + 1],
                                in1=dm[s][:, :], op0=ADD, op1=ADD,
                            )
                            if DEBUG and g == 0 and s == 0:
                                sctf = work.tile([128, N], F32, tag="sctf", bufs=1)
                                nc.vector.tensor_copy(out=sctf[:, :], in_=sc1[:, :])
                                nc.sync.dma_start(out=dbg["sct"][:, :], in_=sctf[:, :])
                            tn1 = work.tile([128, N], BF, tag="tn1")
                            nc.vector.tensor_scalar(
                                out=tn1[:, :], in0=sc1[:, :], scalar1=ALPHA,
                                scalar2=None, op0=MULT,
                            )
                            lr1 = work.tile([128, N], BF, tag="lr1")
                            nc.vector.tensor_tensor(
                                out=lr1[:, :], in0=sc1[:, :], in1=tn1[:, :],
                                op=mybir.AluOpType.max,
                            )
                            ex1 = work.tile([128, N], BF, tag="ex1")
                            z1 = work.tile([128, 1], F32, tag="z1")
                            nc.scalar.activation(
                                ex1[:, :], lr1[:, :], AF.Exp, accum_out=z1[:, :]
                            )
                            rz1 = work.tile([128, 1], F32, tag="rz1")
                            nc.vector.reciprocal(rz1[:, :], z1[:, :])
                            at1 = work.tile([128, N], BF, tag="at1")
                            if s == 1:
                                # head B is computed by two cores; halve it
                                nc.vector.tensor_scalar(
                                    out=at1[:, :], in0=ex1[:, :], scalar1=rz1[:, :],
                                    scalar2=0.5, op0=MULT, op1=MULT,
                                )
                            else:
                                nc.vector.tensor_scalar(
                                    out=at1[:, :], in0=ex1[:, :], scalar1=rz1[:, :],
                                    scalar2=None, op0=MULT,
                                )
                            if DEBUG and g == 0 and s == 0:
                                at1f = work.tile([128, N], F32, tag="at1f", bufs=1)
                                nc.vector.tensor_copy(out=at1f[:, :], in_=at1[:, :])
                                nc.sync.dma_start(out=dbg["at1"][:, :], in_=at1f[:, :])
                            at1T = at1pool.tile([128, 512], BF, tag="at1T")
                            for q in range(4):
                                pt = pst1.tile([128, 128], BF, tag="ptr1")
                                nc.tensor.transpose(
                                    pt[:, :], at1[:, 128 * q : 128 * (q + 1)], ident[:, :]
                                )
                                nc.vector.tensor_copy(
                                    out=at1T[:, 128 * q : 128 * (q + 1)], in_=pt[:, :]
                                )
                            for q in range(4):
                                lhsq = at1T[:, 128 * q : 128 * (q + 1)]
                                nc.tensor.matmul(
                                    poa[:, :], lhsq, h1q[q][:, D * s : D * s + 512],
                                    start=(s == 0 and q == 0), stop=(s == 1 and q == 3),
                                )
                                nc.tensor.matmul(
                                    pob[:, :], lhsq, h1q[q][:, D * s + 512 : D * (s + 1)],
                                    start=(s == 0 and q == 0), stop=(s == 1 and q == 3),
                                )
                        rsst = work.tile([128, D], F32, tag="rsst")
                        nc.vector.tensor_scalar(
                            out=rsst[:, 0:512], in0=poa[:, :], scalar1=1.0 / H,
                            scalar2=None, op0=MULT,
                        )
                        nc.vector.tensor_scalar(
                            out=rsst[:, 512:768], in0=pob[:, :], scalar1=1.0 / H,
                            scalar2=None, op0=MULT,
                        )
                        if DEBUG and g == 0:
                            nc.sync.dma_start(out=dbg["rsst"][:, :], in_=rsst[:, :])
                        nc.sync.dma_start(
                            out=rs_in[128 * g : 128 * (g + 1), :], in_=rsst[:, :]
                        )

                # ---- ReduceScatter partial outputs -> my 64 rows ----
                nc.gpsimd.collective_compute(
                    "ReduceScatter", ADD,
                    replica_groups=groups,
                    ins=[rs_in.ap().opt()], outs=[rs_out.ap().opt()],
                )
                opf = work.tile([64, D], F32, tag="opf", bufs=1)
                nc.sync.dma_start(out=opf[:, :], in_=rs_out[:, :])
                omin = work.tile([64, D], F32, tag="omin", bufs=1)
                nc.vector.tensor_scalar(
                    out=omin[:, :], in0=opf[:, :], scalar1=0.0, scalar2=None,
                    op0=mybir.AluOpType.min,
                )
                oexp = work.tile([64, D], F32, tag="oexp", bufs=1)
                nc.scalar.activation(oexp[:, :], omin[:, :], AF.Exp)
                omax = work.tile([64, D], F32, tag="omax", bufs=1)
                nc.vector.tensor_scalar(
                    out=omax[:, :], in0=opf[:, :], scalar1=0.0, scalar2=None,
                    op0=mybir.AluOpType.max,
                )
                ofin = work.tile([64, D], F32, tag="ofin", bufs=1)
                nc.vector.scalar_tensor_tensor(
                    out=ofin[:, :], in0=oexp[:, :], scalar=-1.0, in1=omax[:, :],
                    op0=ADD, op1=ADD,
                )
                nc.scalar.dma_start(out=out_d[:, :], in_=ofin[:, :])

    nc.compile()
    return nc


def _fold_weights(We, W, a, F_):
    We = We.astype(np.float64)
    W = W.astype(np.float64)
    a = a.astype(np.float64)
    a1, a2, a3 = a[:, :F_], a[:, F_ : 2 * F_], a[:, 2 * F_ :]
    v = np.einsum("hei,hif,hf->he", We, W, a3)
    usrc = np.einsum("hif,hf->hi", W, a1)
    udst = np.einsum("hif,hf->hi", W, a2)
    return v, usrc, udst


def _to_ktile(mat):
    """[768, C] -> [128, KT*C] with the KT k-tiles side by side."""
    k, c = mat.shape
    assert k == D
    return np.ascontiguousarray(
        mat.reshape(KT, 128, c).transpose(1, 0, 2).reshape(128, KT * c)
    )


def kernel(**inputs):
    global _COMPILED
    x = np.asarray(inputs["x"], dtype=np.float32)
    adj = np.asarray(inputs["adj"])
    e = np.asarray(inputs["e"], dtype=np.float32)
    W0 = np.asarray(inputs["W0"], dtype=np.float32)
    a0 = np.asarray(inputs["a0"], dtype=np.float32)
    W1 = np.asarray(inputs["W1"], dtype=np.float32)
    a1_ = np.asarray(inputs["a1"], dtype=np.float32)
    We0 = np.asarray(inputs["We0"], dtype=np.float32)
    We1 = np.asarray(inputs["We1"], dtype=np.float32)

    v0, _, _ = _fold_weights(We0, W0, a0, F0)
    v1, u1src, u1dst = _fold_weights(We1, W1, a1_, D)

    # V slot layout: 0-11 = layer-0 heads; 12+2g+s: s=0 -> head g, s=1 -> head 8+g//2
    V32 = np.zeros((D, 32), np.float64)
    V32[:, :12] = v0.T
    for g in range(NCORES):
        V32[:, 12 + 2 * g] = v1[g]
        V32[:, 12 + 2 * g + 1] = v1[8 + g // 2]
    v_bf = _to_ktile(V32.astype(np.float32)).astype(BF16)

    h0h = np.einsum("ni,hif->hnf", x.astype(np.float64), W0.astype(np.float64))
    s_src0 = np.einsum("hnf,hf->hn", h0h, a0[:, :F0].astype(np.float64))
    s_dst0 = np.einsum("hnf,hf->hn", h0h, a0[:, F0 : 2 * F0].astype(np.float64))
    maskadd = (adj.astype(np.float32) - 1.0) * 9e15                   # 0 or -9e15

    xT_bf = _to_ktile(np.ascontiguousarray(x.T)).astype(BF16)
    w0r_bf = _to_ktile(W0.transpose(1, 0, 2).reshape(D, H * F0)).astype(BF16)
    W1r = W1.transpose(1, 0, 2).reshape(D, H * D)
    ident = np.eye(128, dtype=np.float32).astype(BF16)
    onesel = np.zeros((4, 256), np.float32)
    onesel[2, 0:128] = 1.0
    onesel[3, 128:256] = 1.0
    onesel = onesel.astype(BF16)

    # block-major fp8 e layout: eb[c, b, p, cc, kb, j] = e[64c+4b+cc, j, 128kb+p]
    e8 = e.astype(ENP)                                   # [i, j, k]
    v8 = e8.reshape(NCORES, NBLK, 4, N, KT, 128)          # [c, b, cc, j, kb, p]
    eb = np.ascontiguousarray(v8.transpose(0, 1, 5, 2, 4, 3)).reshape(
        NCORES, NBLK, 128, 24 * N
    )

    in_maps = []
    for c in range(NCORES):
        hA = c
        hB = 8 + c // 2
        ha0 = np.zeros((NBLK, 128, N), dtype=np.float32)
        for b in range(NBLK):
            for cc in range(4):
                i = NPC * c + 4 * b + cc
                ha0[b, 32 * cc : 32 * cc + 12, :] = (
                    s_dst0 + s_src0[:, i : i + 1] + maskadd[i : i + 1, :]
                )
                ha0[b, 32 * cc + 12 : 32 * cc + 28, :] = maskadd[i : i + 1, :]
        w1h_bf = _to_ktile(
            np.ascontiguousarray(
                np.concatenate(
                    [W1r[:, D * hA : D * (hA + 1)], W1r[:, D * hB : D * (hB + 1)]],
                    axis=1,
                )
            )
        ).astype(BF16)
        u1sel = np.stack(
            [u1src[hA], u1src[hB], u1dst[hA], u1dst[hB]], axis=1
        ).astype(np.float32)                                # [768, 4]
        in_maps.append(
            {
                "eT": eb[c],
                "xT": xT_bf,
                "w0r": w0r_bf,
                "w1h": w1h_bf,
                "vw": v_bf,
                "u1sel": _to_ktile(u1sel).astype(BF16),
                "ha0": ha0.astype(BF16),
                "ident": ident,
                "onesel": onesel,
            }
        )

    if _COMPILED is None:
        _COMPILED = _build_nc()
    nc = _COMPILED

    res = run_bass_kernel_spmd(nc, in_maps, list(range(NCORES)))
    global _LAST_RESULTS
    _LAST_RESULTS = res.results
    out = np.concatenate([res.results[c]["out"] for c in range(NCORES)], axis=0)
    return out.astype(np.float32)


if __name__ == "__main__":
    import reference

    inputs = {k: np.asarray(v) for k, v in reference.setup_inputs().items()}
    got = kernel(**inputs)
    print("output shape:", got.shape, got.dtype)


# revision 47
# speedup vs baseline: 1.0412x; 1.0296x over previous
"""Bass/Trainium2 kernel for a 2-layer multi-head GAT (DocRE model).

Contract: kernel(**inputs) takes the FULL unsharded inputs as numpy arrays
and returns the FULL [512, 768] float32 output.

Sharding / dataflow (v2, collective-light):
- Layer 0 is row-sharded: each core streams its 64 rows of the edge tensor e
  once (block-major fp8-e3m4 layout) and computes BOTH layers' edge scores
  s_e = e @ V in a single pass (V holds 12 layer-0 head columns plus 16
  layer-1 "slot" columns = (dst core, slot) pairs).
- Layer-1 edge scores are AllToAll'd to a head-sharded layout DURING the
  e-stream (2 chunks; the first is hidden under the stream). Each core then
  owns 2 layer-1 heads: head A = core id (weight 1), head B = 8 + core//2
  (computed by two cores, each weighted 1/2).
- x1 is AllGathered once (small); h1 columns for the core's two heads are
  computed locally from the gathered x1 -> NO AllGather of the big h1.
- Per-head partial outputs (all 512 rows) are ReduceScatter'd (fp32 add) to
  give each core its 64 output rows. Total collective payload is ~25x
  smaller than the v1 AllGather-h1 scheme.
- Additive score pieces: layer-0 s_src+s_dst+mask and the layer-1 mask are
  folded into one host-precomputed per-block tile (ha0); layer-1 s_dst rides
  a PE outer-product broadcast, s_src rides the LeakyReLU activation bias.
"""

import sys

sys.path.insert(0, "/opt/trn_rl_repo")

import numpy as np
import ml_dtypes

from concourse import bass, bacc, mybir, tile
from concourse.bass_utils import run_bass_kernel_spmd

BF16 = ml_dtypes.bfloat16
FP8 = ml_dtypes.float8_e3m4

N = 512          # nodes
D = 768          # hidden
H = 12           # heads
F0 = 64          # layer-0 per-head dim
NCORES = 8
NPC = N // NCORES          # 64 local rows per core
NBLK = NPC // 4            # 16 blocks of 4 rows
ALPHA = 0.2
KT = D // 128              # 6 contraction tiles
W1H = 2 * D                # h1 columns per core (2 heads)
SPLIT = 12                 # e-blocks in the first AllToAll chunk
NEG = -9e15

E_FP8 = True               # stream e as fp8-e3m4 (else bf16)

F32 = mybir.dt.float32
BF = mybir.dt.bfloat16
EDT = mybir.dt.float8e3 if E_FP8 else mybir.dt.bfloat16
ENP = FP8 if E_FP8 else BF16
ADD = mybir.AluOpType.add
MULT = mybir.AluOpType.mult
AF = mybir.ActivationFunctionType

_COMPILED = None
DEBUG = False
SIM_SAFE = False  # replace Prelu (not in interp) with Relu for cost-model sims
_LAST_RESULTS = None


def _build_nc():
    nc = bacc.Bacc("TRN2", target_bir_lowering=False, num_devices=NCORES)
    lrelu_fn = AF.Relu if SIM_SAFE else AF.Prelu
    NA = 4 * SPLIT               # i-rows per src in AllToAll chunk a
    NB = NPC - NA                # i-rows per src in chunk b
    dbg = {}
    if DEBUG:
        dbg["sc0"] = nc.dram_tensor("dbg_sc0", [128, N], F32, kind="ExternalOutput")
        dbg["at0"] = nc.dram_tensor("dbg_at0", [128, N], F32, kind="ExternalOutput")
        dbg["at0T"] = nc.dram_tensor("dbg_at0T", [128, 128], F32, kind="ExternalOutput")
        dbg["x1p"] = nc.dram_tensor("dbg_x1p", [NPC, D], F32, kind="ExternalOutput")
        dbg["h0"] = nc.dram_tensor("dbg_h0", [128, D], F32, kind="ExternalOutput")
        dbg["x1"] = nc.dram_tensor("dbg_x1", [NPC, D], F32, kind="ExternalOutput")
        dbg["s1sel"] = nc.dram_tensor("dbg_s1sel", [4, N], F32, kind="ExternalOutput")
        dbg["sct"] = nc.dram_tensor("dbg_sct", [128, N], F32, kind="ExternalOutput")
        dbg["at1"] = nc.dram_tensor("dbg_at1", [128, N], F32, kind="ExternalOutput")
        dbg["h1q"] = nc.dram_tensor("dbg_h1q", [128, W1H], F32, kind="ExternalOutput")
        dbg["rsst"] = nc.dram_tensor("dbg_rsst", [128, D], F32, kind="ExternalOutput")

    eT_d = nc.dram_tensor("eT", [NBLK, 128, 24 * N], EDT, kind="ExternalInput")
    xT_d = nc.dram_tensor("xT", [128, KT * N], BF, kind="ExternalInput")
    w0r_d = nc.dram_tensor("w0r", [128, KT * D], BF, kind="ExternalInput")
    w1h_d = nc.dram_tensor("w1h", [128, KT * W1H], BF, kind="ExternalInput")
    v_d = nc.dram_tensor("vw", [128, KT * 32], BF, kind="ExternalInput")
    u1_d = nc.dram_tensor("u1sel", [128, KT * 4], BF, kind="ExternalInput")
    ha0_d = nc.dram_tensor("ha0", [NBLK, 128, N], BF, kind="ExternalInput")
    ident_d = nc.dram_tensor("ident", [128, 128], BF, kind="ExternalInput")
    # onesel[:, 128s:128(s+1)] is a [4,128] selector with row 2+s all-ones:
    # onesel[:, s].T @ s1s[0:4] broadcasts dst row s across 128 partitions.
    ones_d = nc.dram_tensor("onesel", [4, 256], BF, kind="ExternalInput")

    out_d = nc.dram_tensor("out", [NPC, D], F32, kind="ExternalOutput")

    NA1 = NA // 2
    at_in_a1 = nc.dram_tensor("at_in_a1", [NCORES, NA1, 2, N], BF)
    at_out_a1 = nc.dram_tensor("at_out_a1", [NCORES, NA1, 2, N], BF)
    at_in_a2 = nc.dram_tensor("at_in_a2", [NCORES, NA1, 2, N], BF)
    at_out_a2 = nc.dram_tensor("at_out_a2", [NCORES, NA1, 2, N], BF)
    agx_in = [
        nc.dram_tensor(f"agx_in{k}", [NPC // 2, D], BF) for k in range(2)
    ]
    agx_out = [
        nc.dram_tensor(f"agx_out{k}", [N // 2, D], BF, addr_space="Shared")
        for k in range(2)
    ]
    # chunk-b AllToAll payload per dst: [NB, 2, N] scores + [32, D] x1 chunk-1
    # (replicated to every dst -> the AllToAll doubles as the 2nd x1 AllGather)
    RB = NB * 2 * N + 32 * D
    at_in_b = nc.dram_tensor("at_in_b", [NCORES, RB], BF)
    at_out_b = nc.dram_tensor("at_out_b", [NCORES, RB], BF)
    rs_in = nc.dram_tensor("rs_in", [N, D], F32)
    rs_out = nc.dram_tensor("rs_out", [NPC, D], F32)

    groups = [list(range(NCORES))]

    with tile.TileContext(nc) as tc:
        with (
            tc.tile_pool(name="const", bufs=1) as constp,
            tc.tile_pool(name="pers", bufs=1) as pers,
            tc.tile_pool(name="hapool", bufs=3) as hapool,
        ):
            # const loads go on the DVE/PE DMA queues so the sync queue can
            # start streaming e-tiles immediately
            ident = constp.tile([128, 128], BF, tag="ident")
            nc.gpsimd.dma_start(out=ident[:, :], in_=ident_d[:, :])
            ones = constp.tile([4, 256], BF, tag="ones")
            nc.gpsimd.dma_start(out=ones[:, :], in_=ones_d[:, :])
            w1h = constp.tile([128, KT * W1H], BF, tag="w1h")
            nc.gpsimd.dma_start(out=w1h[:, :], in_=w1h_d[:, :])
            u1 = constp.tile([128, KT * 4], BF, tag="u1")
            nc.gpsimd.dma_start(out=u1[:, :], in_=u1_d[:, :])

            x1T = pers.tile([128, KT * N], BF, tag="x1T")

            # =================== phase A: e-stream + layer 0 ===================
            with (
                tc.tile_pool(name="l0pers", bufs=1) as l0p,
                tc.tile_pool(name="l0const", bufs=1) as l0c,
                tc.tile_pool(name="epool", bufs=3) as epool,
                tc.tile_pool(name="l0work", bufs=2) as work,
            ):
                vw = l0c.tile([128, KT * 32], BF, tag="vw")
                nc.sync.dma_start(out=vw[:, :], in_=v_d[:, :])
                xT = l0c.tile([128, KT * N], BF, tag="xT")
                nc.scalar.dma_start(out=xT[:, :], in_=xT_d[:, :])
                w0r = l0c.tile([128, KT * D], BF, tag="w0r")
                nc.scalar.dma_start(out=w0r[:, :], in_=w0r_d[:, :])

                # ---- h0 = x @ W0r -> [4][128 nodes, 768] bf16 ----
                # (emitted inside the e-loop at b==2 so the first e-blocks'
                # matmuls keep the et-DMA pipeline primed)
                h0 = [l0p.tile([128, D], BF, tag=f"h0_{m}", name=f"h0_{m}") for m in range(4)]

                def h0_compute(psh0):
                    for m in range(4):
                        pa = psh0.tile([128, 512], F32, tag="ph0a")
                        pb = psh0.tile([128, 256], F32, tag="ph0b")
                        for k in range(KT):
                            lhs = xT[:, k * N + 128 * m : k * N + 128 * (m + 1)]
                            nc.tensor.matmul(
                                pa[:, :], lhs, w0r[:, k * D : k * D + 512],
                                start=(k == 0), stop=(k == KT - 1),
                            )
                            nc.tensor.matmul(
                                pb[:, :], lhs, w0r[:, k * D + 512 : (k + 1) * D],
                                start=(k == 0), stop=(k == KT - 1),
                            )
                        nc.vector.tensor_copy(out=h0[m][:, 0:512], in_=pa[:, :])
                        nc.vector.tensor_copy(out=h0[m][:, 512:768], in_=pb[:, :])
                    if DEBUG:
                        h0f = l0p.tile([128, D], F32, tag="h0f")
                        nc.vector.tensor_copy(out=h0f[:, :], in_=h0[0][:, :])
                        nc.sync.dma_start(out=dbg["h0"][:, :], in_=h0f[:, :])

                # ---- e-pass: scores, L1 staging, L0 softmax, att0^T ----
                at0T = [
                    l0p.tile([128, NBLK * 128], BF, tag=f"at0T_{q}", name=f"at0T_{q}") for q in range(4)
                ]
                with (
                    tc.tile_pool(name="psv", bufs=2, space="PSUM") as psvp,
                    tc.tile_pool(name="pst", bufs=2, space="PSUM") as pstp,
                    tc.tile_pool(name="psx1", bufs=1, space="PSUM") as psx1,
                    tc.tile_pool(name="psh0", bufs=1, space="PSUM") as psh0,
                ):

                    def x1_chunk(k):
                        """x1 rows [32k, 32k+32) = elu(att0 @ h0), then AllGather."""
                        px1a = psx1.tile([32, 512], F32, tag="px1a")
                        px1b = psx1.tile([32, 256], F32, tag="px1b")
                        for h in range(H):
                            dsti = (
                                px1a[:, 64 * h : 64 * (h + 1)]
                                if h < 8
                                else px1b[:, 64 * (h - 8) : 64 * (h - 7)]
                            )
                            for q in range(4):
                                lhs = at0T[q][:, :].rearrange(
                                    "p (b c r) -> p b c r", b=NBLK, c=4
                                )[:, 8 * k : 8 * k + 8, :, h : h + 1]
                                nc.tensor.matmul(
                                    dsti, lhs, h0[q][:, 64 * h : 64 * (h + 1)],
                                    start=(q == 0), stop=(q == 3),
                                )
                        x1p = work.tile([32, D], F32, tag="x1p")
                        nc.vector.tensor_copy(out=x1p[:, 0:512], in_=px1a[:, :])
                        nc.vector.tensor_copy(out=x1p[:, 512:768], in_=px1b[:, :])
                        tmin = work.tile([32, D], F32, tag="tmin")
                        nc.vector.tensor_scalar(
                            out=tmin[:, :], in0=x1p[:, :], scalar1=0.0, scalar2=None,
                            op0=mybir.AluOpType.min,
                        )
                        texp = work.tile([32, D], F32, tag="texp")
                        nc.scalar.activation(texp[:, :], tmin[:, :], AF.Exp)
                        tmax = work.tile([32, D], F32, tag="tmax")
                        nc.vector.tensor_scalar(
                            out=tmax[:, :], in0=x1p[:, :], scalar1=0.0, scalar2=None,
                            op0=mybir.AluOpType.max,
                        )
                        x1bf = work.tile([32, D], BF, tag="x1bf")
                        nc.vector.scalar_tensor_tensor(
                            out=x1bf[:, :], in0=texp[:, :], scalar=-1.0, in1=tmax[:, :],
                            op0=ADD, op1=ADD,
                        )
                        if DEBUG:
                            x1f32 = work.tile([32, D], F32, tag="x1f32")
                            nc.vector.scalar_tensor_tensor(
                                out=x1f32[:, :], in0=texp[:, :], scalar=-1.0,
                                in1=tmax[:, :], op0=ADD, op1=ADD,
                            )
                            nc.sync.dma_start(
                                out=dbg["x1"][32 * k : 32 * (k + 1), :], in_=x1f32[:, :]
                            )
                        if k == 0:
                            nc.scalar.dma_start(out=agx_in[k][:, :], in_=x1bf[:, :])
                            nc.gpsimd.collective_compute(
                                "AllGather", mybir.AluOpType.bypass,
                                replica_groups=groups,
                                ins=[agx_in[k].ap().opt()], outs=[agx_out[k].ap().opt()],
                            )
                        else:
                            # replicate x1 chunk-1 into every dst's AllToAll
                            # block: AT_b doubles as the 2nd x1 AllGather.
                            for g in range(NCORES):
                                eng = nc.scalar if g % 2 == 0 else nc.sync
                                eng.dma_start(
                                    out=at_in_b[
                                        g, NB * 2 * N : NB * 2 * N + 32 * D
                                    ].rearrange("(r f) -> r f", r=32),
                                    in_=x1bf[:, :],
                                )

                    for b in range(NBLK):
                        if b == 2:
                            h0_compute(psh0)
                        et = epool.tile([128, 24 * N], EDT, tag="etile")
                        nc.sync.dma_start(out=et[:, :], in_=eT_d[b])
                        ha = hapool.tile([128, N], BF, tag="ha0")
                        nc.scalar.dma_start(out=ha[:, :], in_=ha0_d[b])

                        psv = psvp.tile([128, N], F32, tag="psv")
                        for cc in range(4):
                            for kb in range(KT):
                                nc.tensor.matmul(
                                    psv[32 * cc : 32 * cc + 32, :],
                                    vw[:, 32 * kb : 32 * (kb + 1)],
                                    et[:, (cc * KT + kb) * N : (cc * KT + kb + 1) * N],
                                    start=(kb == 0), stop=(kb == KT - 1),
                                    tile_position=(0, 32 * cc),
                                )
                        # full scores (L0 rows 0:12, L1 rows 12:28 per cc-group)
                        sc0 = work.tile([128, N], BF, tag="sc0")
                        nc.vector.tensor_tensor(
                            out=sc0[:, :], in0=psv[:, :], in1=ha[:, :], op=ADD
                        )
                        if DEBUG and b == 0:
                            sc0f = work.tile([128, N], F32, tag="sc0f", bufs=1)
                            nc.vector.tensor_copy(out=sc0f[:, :], in_=sc0[:, :])
                            nc.sync.dma_start(out=dbg["sc0"][:, :], in_=sc0f[:, :])
                        # stage L1 rows to the AllToAll input buffer.
                        # NOTE: one DMA per cc-group — split-partition rearrange
                        # APs on SBUF tiles break tile dependency tracking.
                        for cc in range(4):
                            if b < SPLIT // 2:
                                dst = at_in_a1[:, 4 * b + cc, :, :]
                            elif b < SPLIT:
                                dst = at_in_a2[:, 4 * (b - SPLIT // 2) + cc, :, :]
                            else:
                                off = (4 * (b - SPLIT) + cc) * 2 * N
                                dst = at_in_b[:, off : off + 2 * N].rearrange(
                                    "g (s j) -> g s j", s=2, j=N
                                )
                            # gpsimd only BEFORE the first collective
                            # (collectives block that queue for their whole
                            # duration); otherwise the Act queue.
                            if cc < 2:
                                eng = nc.scalar
                            elif b < 5:
                                eng = nc.gpsimd
                            else:
                                eng = nc.scalar
                            eng.dma_start(
                                out=dst,
                                in_=sc0[32 * cc + 12 : 32 * cc + 28, :],
                            )
                        # layer-0 softmax (valid rows cc*32+[0:12); rest harmless)
                        # LeakyReLU on DVE as max(x, 0.2x) keeps the Activation
                        # engine Exp-only: no act-table reloads.
                        tn0 = work.tile([128, N], BF, tag="tn0")
                        nc.vector.tensor_scalar(
                            out=tn0[:, :], in0=sc0[:, :], scalar1=ALPHA, scalar2=None,
                            op0=MULT,
                        )
                        lr0 = work.tile([128, N], BF, tag="lr0")
                        nc.vector.tensor_tensor(
                            out=lr0[:, :], in0=sc0[:, :], in1=tn0[:, :],
                            op=mybir.AluOpType.max,
                        )
                        ex0 = work.tile([128, N], BF, tag="ex0")
                        z0 = work.tile([128, 1], F32, tag="z0")
                        nc.scalar.activation(
                            ex0[:, :], lr0[:, :], AF.Exp, accum_out=z0[:, :]
                        )
                        rz0 = work.tile([128, 1], F32, tag="rz0")
                        nc.vector.reciprocal(rz0[:, :], z0[:, :])
                        at0 = work.tile([128, N], BF, tag="at0")
                        nc.vector.tensor_scalar(
                            out=at0[:, :], in0=ex0[:, :], scalar1=rz0[:, :],
                            scalar2=None, op0=MULT,
                        )
                        if DEBUG and b == 0:
                            at0f = work.tile([128, N], F32, tag="at0f", bufs=1)
                            nc.vector.tensor_copy(out=at0f[:, :], in_=at0[:, :])
                            nc.sync.dma_start(out=dbg["at0"][:, :], in_=at0f[:, :])
                        for q in range(4):
                            pt = pstp.tile([128, 128], BF, tag="ptr")
                            nc.tensor.transpose(
                                pt[:, :], at0[:, 128 * q : 128 * (q + 1)], ident[:, :]
                            )
                            nc.vector.tensor_copy(
                                out=at0T[q][:, 128 * b : 128 * (b + 1)], in_=pt[:, :]
                            )
                        if b == SPLIT // 2 - 1:
                            nc.gpsimd.collective_compute(
                                "AllToAll", mybir.AluOpType.bypass,
                                replica_groups=groups,
                                ins=[at_in_a1.ap().opt()], outs=[at_out_a1.ap().opt()],
                            )
                        if b == SPLIT - 1:
                            nc.gpsimd.collective_compute(
                                "AllToAll", mybir.AluOpType.bypass,
                                replica_groups=groups,
                                ins=[at_in_a2.ap().opt()], outs=[at_out_a2.ap().opt()],
                            )
                        if b == 7:
                            x1_chunk(0)
                    x1_chunk(1)

                # second AllToAll chunk (ready at stream end)
                nc.gpsimd.collective_compute(
                    "AllToAll", mybir.AluOpType.bypass,
                    replica_groups=groups,
                    ins=[at_in_b.ap().opt()], outs=[at_out_b.ap().opt()],
                )
                with tc.tile_pool(name="x1fp", bufs=1) as x1fp:
                    x1f = [x1fp.tile([128, D], BF, tag=f"x1f_{m}", name=f"x1f_{m}") for m in range(4)]
                    for m in range(4):
                        # global row j = 64*src + 32*k + r -> partition 64*ds+32*k+r
                        for ds in range(2):
                            nc.sync.dma_start(
                                out=x1f[m][64 * ds : 64 * ds + 32, :],
                                in_=agx_out[0][
                                    32 * (2 * m + ds) : 32 * (2 * m + ds) + 32, :
                                ],
                            )
                            nc.sync.dma_start(
                                out=x1f[m][64 * ds + 32 : 64 * ds + 64, :],
                                in_=at_out_b[
                                    2 * m + ds, NB * 2 * N : NB * 2 * N + 32 * D
                                ].rearrange("(r f) -> r f", r=32),
                            )
                    with tc.tile_pool(name="psxt", bufs=2, space="PSUM") as psxt:
                        for m in range(4):
                            for k6 in range(KT):
                                pt = psxt.tile([128, 128], BF, tag="pxt")
                                nc.tensor.transpose(
                                    pt[:, :],
                                    x1f[m][:, 128 * k6 : 128 * (k6 + 1)],
                                    ident[:, :],
                                )
                                nc.vector.tensor_copy(
                                    out=x1T[
                                        :, N * k6 + 128 * m : N * k6 + 128 * (m + 1)
                                    ],
                                    in_=pt[:, :],
                                )

            # =================== tail: layer 1, head-sharded ===================
            with (
                tc.tile_pool(name="l1pers", bufs=1) as l1p,
                tc.tile_pool(name="l1work", bufs=2) as work,
                tc.tile_pool(name="scpool", bufs=3) as scpool,
                tc.tile_pool(name="at1pool", bufs=2) as at1pool,
            ):
                # s1sel = [src_A, src_B, dst_A, dst_B]^T [4, N]
                s1s = l1p.tile([4, N], BF, tag="s1s")
                srcT = l1p.tile([128, 8], BF, tag="srcT")  # [i, 2s] per g pair cols
                dm = [l1p.tile([128, N], BF, tag=f"dm_{s}", name=f"dm_{s}") for s in range(2)]
                with tc.tile_pool(name="pss1", bufs=1, space="PSUM") as pss1:
                    ps1 = pss1.tile([4, N], F32, tag="ps1")
                    for k in range(KT):
                        nc.tensor.matmul(
                            ps1[:, :], u1[:, 4 * k : 4 * (k + 1)],
                            x1T[:, N * k : N * (k + 1)],
                            start=(k == 0), stop=(k == KT - 1),
                        )
                    nc.vector.tensor_copy(out=s1s[:, :], in_=ps1[:, :])
                    if DEBUG:
                        s1f = work.tile([4, N], F32, tag="s1f", bufs=1)
                        nc.vector.tensor_copy(out=s1f[:, :], in_=ps1[:, :])
                        nc.sync.dma_start(out=dbg["s1sel"][:, :], in_=s1f[:, :])
                with tc.tile_pool(name="psdm", bufs=1, space="PSUM") as psdm:
                    # srcT[:, 2g+s] = s1sel[s, 128g:128(g+1)]
                    for g in range(4):
                        pt4 = psdm.tile([128, 2], BF, tag="pt4")
                        nc.tensor.transpose(
                            pt4[:, :], s1s[0:2, 128 * g : 128 * (g + 1)], ident[0:2, 0:2]
                        )
                        nc.vector.tensor_copy(
                            out=srcT[:, 2 * g : 2 * g + 2], in_=pt4[:, :]
                        )
                    # dm[s] = broadcast of dst row s over 128 partitions
                    for s in range(2):
                        pdm = psdm.tile([128, N], F32, tag="pdm")
                        nc.tensor.matmul(
                            pdm[:, :], ones[:, 128 * s : 128 * (s + 1)], s1s[0:4, :],
                            start=True, stop=True,
                        )
                        nc.vector.tensor_copy(out=dm[s][:, :], in_=pdm[:, :])

                # ---- h1 for my 2 heads: [4 jq][128, W1H] ----
                h1q = [l1p.tile([128, W1H], BF, tag=f"h1q_{q}", name=f"h1q_{q}") for q in range(4)]
                widths = [(0, 512), (512, 1024), (1024, 1536)]
                with tc.tile_pool(name="psh1", bufs=2, space="PSUM") as psh1:
                    for m in range(4):
                        ph1 = [
                            psh1.tile([128, 512], F32, tag="ph1a", name="ph1a"),
                            psh1.tile([128, 512], F32, tag="ph1b", name="ph1b"),
                            psh1.tile([128, 512], F32, tag="ph1c", name="ph1c"),
                        ]
                        for k in range(KT):
                            lhs = x1T[:, N * k + 128 * m : N * k + 128 * (m + 1)]
                            for t, (c0, c1) in enumerate(widths):
                                nc.tensor.matmul(
                                    ph1[t][:, 0 : c1 - c0], lhs,
                                    w1h[:, W1H * k + c0 : W1H * k + c1],
                                    start=(k == 0), stop=(k == KT - 1),
                                )
                        for t, (c0, c1) in enumerate(widths):
                            nc.scalar.copy(
                                out=h1q[m][:, c0:c1], in_=ph1[t][:, 0 : c1 - c0]
                            )

                # ---- per-igroup: softmax for both heads, att @ h1, partials ----
                with (
                    tc.tile_pool(name="pst1", bufs=2, space="PSUM") as pst1,
                    tc.tile_pool(name="pso", bufs=2, space="PSUM") as psop,
                ):
                    for g in range(4):
                        poa = psop.tile([128, 512], F32, tag="poa")
                        pob = psop.tile([128, 256], F32, tag="pob")
                        for s in range(2):
                            sct = scpool.tile([128, N], BF, tag="sct")
                            for ds in range(2):
                                nc.sync.dma_start(
                                    out=sct[64 * ds : 64 * ds + 2 * SPLIT, :],
                                    in_=at_out_a1[2 * g + ds, :, s, :],
                                )
                                nc.sync.dma_start(
                                    out=sct[64 * ds + 2 * SPLIT : 64 * ds + 4 * SPLIT, :],
                                    in_=at_out_a2[2 * g + ds, :, s, :],
                                )
                                nc.sync.dma_start(
                                    out=sct[64 * ds + 4 * SPLIT : 64 * (ds + 1), :],
                                    in_=at_out_b[
                                        2 * g + ds, 0 : NB * 2 * N
                                    ].rearrange("(i s j) -> i s j", s=2, j=N)[:, s, :],
                                )
                            sc1 = work.tile([128, N], BF, tag="sc1")
                            nc.vector.scalar_tensor_tensor(
                                out=sc1[:, :], in0=sct[:, :],
                                scalar=srcT[:, 2 * g + s : 2 * g + s + 1],
                                in1=dm[s][:, :], op0=ADD, op1=ADD,
                            )
                            if DEBUG and g == 0 and s == 0:
                                sctf = work.tile([128, N], F32, tag="sctf", bufs=1)
                                nc.vector.tensor_copy(out=sctf[:, :], in_=sc1[:, :])
                                nc.sync.dma_start(out=dbg["sct"][:, :], in_=sctf[:, :])
                            tn1 = work.tile([128, N], BF, tag="tn1")
                            nc.vector.tensor_scalar(
                                out=tn1[:, :], in0=sc1[:, :], scalar1=ALPHA,
                                scalar2=None, op0=MULT,
                            )
                            lr1 = work.tile([128, N], BF, tag="lr1")
                            nc.vector.tensor_tensor(
                                out=lr1[:, :], in0=sc1[:, :], in1=tn1[:, :],
                                op=mybir.AluOpType.max,
                            )
                            ex1 = work.tile([128, N], BF, tag="ex1")
                            z1 = work.tile([128, 1], F32, tag="z1")
                            nc.scalar.activation(
                                ex1[:, :], lr1[:, :], AF.Exp, accum_out=z1[:, :]
                            )
                            rz1 = work.tile([128, 1], F32, tag="rz1")
                            nc.vector.reciprocal(rz1[:, :], z1[:, :])
                            at1 = work.tile([128, N], BF, tag="at1")
                            if s == 1:
                                # head B is computed by two cores; halve it
                                nc.vector.tensor_scalar(
                                    out=at1[:, :], in0=ex1[:, :], scalar1=rz1[:, :],
                                    scalar2=0.5, op0=MULT, op1=MULT,
                                )
                            else:
                                nc.vector.tensor_scalar(
                                    out=at1[:, :], in0=ex1[:, :], scalar1=rz1[:, :],
                                    scalar2=None, op0=MULT,
                                )
                            if DEBUG and g == 0 and s == 0:
                                at1f = work.tile([128, N], F32, tag="at1f", bufs=1)
                                nc.vector.tensor_copy(out=at1f[:, :], in_=at1[:, :])
                                nc.sync.dma_start(out=dbg["at1"][:, :], in_=at1f[:, :])
                            at1T = at1pool.tile([128, 512], BF, tag="at1T")
                            for q in range(4):
                                pt = pst1.tile([128, 128], BF, tag="ptr1")
                                nc.tensor.transpose(
                                    pt[:, :], at1[:, 128 * q : 128 * (q + 1)], ident[:, :]
                                )
                                nc.vector.tensor_copy(
                                    out=at1T[:, 128 * q : 128 * (q + 1)], in_=pt[:, :]
                                )
                            for q in range(4):
                                lhsq = at1T[:, 128 * q : 128 * (q + 1)]
                                nc.tensor.matmul(
                                    poa[:, :], lhsq, h1q[q][:, D * s : D * s + 512],
                                    start=(s == 0 and q == 0), stop=(s == 1 and q == 3),
                                )
                                nc.tensor.matmul(
                                    pob[:, :], lhsq, h1q[q][:, D * s + 512 : D * (s + 1)],
                                    start=(s == 0 and q == 0), stop=(s == 1 and q == 3),
                                )
                        rsst = work.tile([128, D], F32, tag="rsst")
                        nc.vector.tensor_scalar(
                            out=rsst[:, 0:512], in0=poa[:, :], scalar1=1.0 / H,
                            scalar2=None, op0=MULT,
                        )
                        nc.vector.tensor_scalar(
                            out=rsst[:, 512:768], in0=pob[:, :], scalar1=1.0 / H,
                            scalar2=None, op0=MULT,
                        )
                        if DEBUG and g == 0:
                            nc.sync.dma_start(out=dbg["rsst"][:, :], in_=rsst[:, :])
                        nc.sync.dma_start(
                            out=rs_in[128 * g : 128 * (g + 1), :], in_=rsst[:, :]
                        )

                # ---- ReduceScatter partial outputs -> my 64 rows ----
                nc.gpsimd.collective_compute(
                    "ReduceScatter", ADD,
                    replica_groups=groups,
                    ins=[rs_in.ap().opt()], outs=[rs_out.ap().opt()],
                )
                opf = work.tile([64, D], F32, tag="opf", bufs=1)
                nc.sync.dma_start(out=opf[:, :], in_=rs_out[:, :])
                omin = work.tile([64, D], F32, tag="omin", bufs=1)
                nc.vector.tensor_scalar(
                    out=omin[:, :], in0=opf[:, :], scalar1=0.0, scalar2=None,
                    op0=mybir.AluOpType.min,
                )
                oexp = work.tile([64, D], F32, tag="oexp", bufs=1)
                nc.scalar.activation(oexp[:, :], omin[:, :], AF.Exp)
                omax = work.tile([64, D], F32, tag="omax", bufs=1)
                nc.vector.tensor_scalar(
                    out=omax[:, :], in0=opf[:, :], scalar1=0.0, scalar2=None,
                    op0=mybir.AluOpType.max,
                )
                ofin = work.tile([64, D], F32, tag="ofin", bufs=1)
                nc.vector.scalar_tensor_tensor(
                    out=ofin[:, :], in0=oexp[:, :], scalar=-1.0, in1=omax[:, :],
                    op0=ADD, op1=ADD,
                )
                nc.scalar.dma_start(out=out_d[:, :], in_=ofin[:, :])

    nc.compile()
    return nc


def _fold_weights(We, W, a, F_):
    We = We.astype(np.float64)
    W = W.astype(np.float64)
    a = a.astype(np.float64)
    a1, a2, a3 = a[:, :F_], a[:, F_ : 2 * F_], a[:, 2 * F_ :]
    v = np.einsum("hei,hif,hf->he", We, W, a3)
    usrc = np.einsum("hif,hf->hi", W, a1)
    udst = np.einsum("hif,hf->hi", W, a2)
    return v, usrc, udst


def _to_ktile(mat):
    """[768, C] -> [128, KT*C] with the KT k-tiles side by side."""
    k, c = mat.shape
    assert k == D
    return np.ascontiguousarray(
        mat.reshape(KT, 128, c).transpose(1, 0, 2).reshape(128, KT * c)
    )


def kernel(**inputs):
    global _COMPILED
    x = np.asarray(inputs["x"], dtype=np.float32)
    adj = np.asarray(inputs["adj"])
    e = np.asarray(inputs["e"], dtype=np.float32)
    W0 = np.asarray(inputs["W0"], dtype=np.float32)
    a0 = np.asarray(inputs["a0"], dtype=np.float32)
    W1 = np.asarray(inputs["W1"], dtype=np.float32)
    a1_ = np.asarray(inputs["a1"], dtype=np.float32)
    We0 = np.asarray(inputs["We0"], dtype=np.float32)
    We1 = np.asarray(inputs["We1"], dtype=np.float32)

    v0, _, _ = _fold_weights(We0, W0, a0, F0)
    v1, u1src, u1dst = _fold_weights(We1, W1, a1_, D)

    # V slot layout: 0-11 = layer-0 heads; 12+2g+s: s=0 -> head g, s=1 -> head 8+g//2
    V32 = np.zeros((D, 32), np.float64)
    V32[:, :12] = v0.T
    for g in range(NCORES):
        V32[:, 12 + 2 * g] = v1[g]
        V32[:, 12 + 2 * g + 1] = v1[8 + g // 2]
    v_bf = _to_ktile(V32.astype(np.float32)).astype(BF16)

    h0h = np.einsum("ni,hif->hnf", x.astype(np.float64), W0.astype(np.float64))
    s_src0 = np.einsum("hnf,hf->hn", h0h, a0[:, :F0].astype(np.float64))
    s_dst0 = np.einsum("hnf,hf->hn", h0h, a0[:, F0 : 2 * F0].astype(np.float64))
    maskadd = (adj.astype(np.float32) - 1.0) * 9e15                   # 0 or -9e15

    xT_bf = _to_ktile(np.ascontiguousarray(x.T)).astype(BF16)
    w0r_bf = _to_ktile(W0.transpose(1, 0, 2).reshape(D, H * F0)).astype(BF16)
    W1r = W1.transpose(1, 0, 2).reshape(D, H * D)
    ident = np.eye(128, dtype=np.float32).astype(BF16)
    onesel = np.zeros((4, 256), np.float32)
    onesel[2, 0:128] = 1.0
    onesel[3, 128:256] = 1.0
    onesel = onesel.astype(BF16)

    # block-major fp8 e layout: eb[c, b, p, cc, kb, j] = e[64c+4b+cc, j, 128kb+p]
    e8 = e.astype(ENP)                                   # [i, j, k]
    v8 = e8.reshape(NCORES, NBLK, 4, N, KT, 128)          # [c, b, cc, j, kb, p]
    eb = np.ascontiguousarray(v8.transpose(0, 1, 5, 2, 4, 3)).reshape(
        NCORES, NBLK, 128, 24 * N
    )

    in_maps = []
    for c in range(NCORES):
        hA = c
        hB = 8 + c // 2
        ha0 = np.zeros((NBLK, 128, N), dtype=np.float32)
        for b in range(NBLK):
            for cc in range(4):
                i = NPC * c + 4 * b + cc
                ha0[b, 32 * cc : 32 * cc + 12, :] = (
                    s_dst0 + s_src0[:, i : i + 1] + maskadd[i : i + 1, :]
                )
                ha0[b, 32 * cc + 12 : 32 * cc + 28, :] = maskadd[i : i + 1, :]
        w1h_bf = _to_ktile(
            np.ascontiguousarray(
                np.concatenate(
                    [W1r[:, D * hA : D * (hA + 1)], W1r[:, D * hB : D * (hB + 1)]],
                    axis=1,
                )
            )
        ).astype(BF16)
        u1sel = np.stack(
            [u1src[hA], u1src[hB], u1dst[hA], u1dst[hB]], axis=1
        ).astype(np.float32)                                # [768, 4]
        in_maps.append(
            {
                "eT": eb[c],
                "xT": xT_bf,
                "w0r": w0r_bf,
                "w1h": w1h_bf,
                "vw": v_bf,
                "u1sel": _to_ktile(u1sel).astype(BF16),
                "ha0": ha0.astype(BF16),
                "ident": ident,
                "onesel": onesel,
            }
        )

    if _COMPILED is None:
        _COMPILED = _build_nc()
    nc = _COMPILED

    res = run_bass_kernel_spmd(nc, in_maps, list(range(NCORES)))
    global _LAST_RESULTS
    _LAST_RESULTS = res.results
    out = np.concatenate([res.results[c]["out"] for c in range(NCORES)], axis=0)
    return out.astype(np.float32)


if __name__ == "__main__":
    import reference

    inputs = {k: np.asarray(v) for k, v in reference.setup_inputs().items()}
    got = kernel(**inputs)
    print("output shape:", got.shape, got.dtype)


# revision 51
# speedup vs baseline: 1.0529x; 1.0112x over previous
"""Bass/Trainium2 kernel for a 2-layer multi-head GAT (DocRE model).

Contract: kernel(**inputs) takes the FULL unsharded inputs as numpy arrays
and returns the FULL [512, 768] float32 output.

Sharding / dataflow (v2, collective-light):
- Layer 0 is row-sharded: each core streams its 64 rows of the edge tensor e
  once (block-major fp8-e3m4 layout) and computes BOTH layers' edge scores
  s_e = e @ V in a single pass (V holds 12 layer-0 head columns plus 16
  layer-1 "slot" columns = (dst core, slot) pairs).
- Layer-1 edge scores are AllToAll'd to a head-sharded layout DURING the
  e-stream (2 chunks; the first is hidden under the stream). Each core then
  owns 2 layer-1 heads: head A = core id (weight 1), head B = 8 + core//2
  (computed by two cores, each weighted 1/2).
- x1 is AllGathered once (small); h1 columns for the core's two heads are
  computed locally from the gathered x1 -> NO AllGather of the big h1.
- Per-head partial outputs (all 512 rows) are ReduceScatter'd (fp32 add) to
  give each core its 64 output rows. Total collective payload is ~25x
  smaller than the v1 AllGather-h1 scheme.
- Additive score pieces: layer-0 s_src+s_dst+mask and the layer-1 mask are
  folded into one host-precomputed per-block tile (ha0); layer-1 s_dst rides
  a PE outer-product broadcast, s_src rides the LeakyReLU activation bias.
"""

import sys

sys.path.insert(0, "/opt/trn_rl_repo")

import numpy as np
import ml_dtypes

from concourse import bass, bacc, mybir, tile
from concourse.bass_utils import run_bass_kernel_spmd

BF16 = ml_dtypes.bfloat16
FP8 = ml_dtypes.float8_e3m4

N = 512          # nodes
D = 768          # hidden
H = 12           # heads
F0 = 64          # layer-0 per-head dim
NCORES = 8
NPC = N // NCORES          # 64 local rows per core
NBLK = NPC // 4            # 16 blocks of 4 rows
ALPHA = 0.2
KT = D // 128              # 6 contraction tiles
W1H = 2 * D                # h1 columns per core (2 heads)
SPLIT = 12                 # e-blocks in the first AllToAll chunk
NEG = -9e15

E_FP8 = True               # stream e as fp8-e3m4 (else bf16)

F32 = mybir.dt.float32
BF = mybir.dt.bfloat16
EDT = mybir.dt.float8e3 if E_FP8 else mybir.dt.bfloat16
ENP = FP8 if E_FP8 else BF16
ADD = mybir.AluOpType.add
MULT = mybir.AluOpType.mult
AF = mybir.ActivationFunctionType

_COMPILED = None
DEBUG = False
SIM_SAFE = False  # replace Prelu (not in interp) with Relu for cost-model sims
_LAST_RESULTS = None


def _build_nc():
    nc = bacc.Bacc("TRN2", target_bir_lowering=False, num_devices=NCORES)
    lrelu_fn = AF.Relu if SIM_SAFE else AF.Prelu
    NA = 4 * SPLIT               # i-rows per src in AllToAll chunk a
    NB = NPC - NA                # i-rows per src in chunk b
    dbg = {}
    if DEBUG:
        dbg["sc0"] = nc.dram_tensor("dbg_sc0", [128, N], F32, kind="ExternalOutput")
        dbg["at0"] = nc.dram_tensor("dbg_at0", [128, N], F32, kind="ExternalOutput")
        dbg["at0T"] = nc.dram_tensor("dbg_at0T", [128, 128], F32, kind="ExternalOutput")
        dbg["x1p"] = nc.dram_tensor("dbg_x1p", [NPC, D], F32, kind="ExternalOutput")
        dbg["h0"] = nc.dram_tensor("dbg_h0", [128, D], F32, kind="ExternalOutput")
        dbg["x1"] = nc.dram_tensor("dbg_x1", [NPC, D], F32, kind="ExternalOutput")
        dbg["s1sel"] = nc.dram_tensor("dbg_s1sel", [4, N], F32, kind="ExternalOutput")
        dbg["sct"] = nc.dram_tensor("dbg_sct", [128, N], F32, kind="ExternalOutput")
        dbg["at1"] = nc.dram_tensor("dbg_at1", [128, N], F32, kind="ExternalOutput")
        dbg["h1q"] = nc.dram_tensor("dbg_h1q", [128, W1H], F32, kind="ExternalOutput")
        dbg["rsst"] = nc.dram_tensor("dbg_rsst", [128, D], F32, kind="ExternalOutput")

    eT_d = nc.dram_tensor("eT", [NBLK, 128, 24 * N], EDT, kind="ExternalInput")
    xT_d = nc.dram_tensor("xT", [128, KT * N], BF, kind="ExternalInput")
    w0r_d = nc.dram_tensor("w0r", [128, KT * D], BF, kind="ExternalInput")
    w1h_d = nc.dram_tensor("w1h", [128, KT * W1H], BF, kind="ExternalInput")
    v_d = nc.dram_tensor("vw", [128, KT * 32], BF, kind="ExternalInput")
    u1_d = nc.dram_tensor("u1sel", [128, KT * 4], BF, kind="ExternalInput")
    ha0_d = nc.dram_tensor("ha0", [NBLK, 128, N], BF, kind="ExternalInput")
    ident_d = nc.dram_tensor("ident", [128, 128], BF, kind="ExternalInput")
    # onesel[:, 128s:128(s+1)] is a [4,128] selector with row 2+s all-ones:
    # onesel[:, s].T @ s1s[0:4] broadcasts dst row s across 128 partitions.
    ones_d = nc.dram_tensor("onesel", [4, 256], BF, kind="ExternalInput")

    out_d = nc.dram_tensor("out", [NPC, D], F32, kind="ExternalOutput")

    NA1 = NA // 2
    at_in_a1 = nc.dram_tensor("at_in_a1", [NCORES, NA1, 2, N], BF)
    at_out_a1 = nc.dram_tensor("at_out_a1", [NCORES, NA1, 2, N], BF)
    at_in_a2 = nc.dram_tensor("at_in_a2", [NCORES, NA1, 2, N], BF)
    at_out_a2 = nc.dram_tensor("at_out_a2", [NCORES, NA1, 2, N], BF)
    agx_in = [
        nc.dram_tensor(f"agx_in{k}", [NPC // 2, D], BF) for k in range(2)
    ]
    agx_out = [
        nc.dram_tensor(f"agx_out{k}", [N // 2, D], BF, addr_space="Shared")
        for k in range(2)
    ]
    # chunk-b AllToAll payload per dst: [NB, 2, N] scores + [32, D] x1 chunk-1
    # (replicated to every dst -> the AllToAll doubles as the 2nd x1 AllGather)
    RB = NB * 2 * N + 32 * D
    at_in_b = nc.dram_tensor("at_in_b", [NCORES, RB], BF)
    at_out_b = nc.dram_tensor("at_out_b", [NCORES, RB], BF)
    rs_in = nc.dram_tensor("rs_in", [N, D], F32)
    rs_out = nc.dram_tensor("rs_out", [NPC, D], F32)

    groups = [list(range(NCORES))]

    with tile.TileContext(nc) as tc:
        with (
            tc.tile_pool(name="const", bufs=1) as constp,
            tc.tile_pool(name="pers", bufs=1) as pers,
            tc.tile_pool(name="hapool", bufs=3) as hapool,
        ):
            # const loads go on the DVE/PE DMA queues so the sync queue can
            # start streaming e-tiles immediately
            ident = constp.tile([128, 128], BF, tag="ident")
            nc.gpsimd.dma_start(out=ident[:, :], in_=ident_d[:, :])
            ones = constp.tile([4, 256], BF, tag="ones")
            nc.gpsimd.dma_start(out=ones[:, :], in_=ones_d[:, :])
            w1h = constp.tile([128, KT * W1H], BF, tag="w1h")
            nc.gpsimd.dma_start(out=w1h[:, :], in_=w1h_d[:, :])
            u1 = constp.tile([128, KT * 4], BF, tag="u1")
            nc.gpsimd.dma_start(out=u1[:, :], in_=u1_d[:, :])

            x1T = pers.tile([128, KT * N], BF, tag="x1T")

            # =================== phase A: e-stream + layer 0 ===================
            with (
                tc.tile_pool(name="l0pers", bufs=1) as l0p,
                tc.tile_pool(name="l0const", bufs=1) as l0c,
                tc.tile_pool(name="epool", bufs=3) as epool,
                tc.tile_pool(name="l0work", bufs=2) as work,
            ):
                vw = l0c.tile([128, KT * 32], BF, tag="vw")
                nc.sync.dma_start(out=vw[:, :], in_=v_d[:, :])
                xT = l0c.tile([128, KT * N], BF, tag="xT")
                nc.scalar.dma_start(out=xT[:, :], in_=xT_d[:, :])
                w0r = l0c.tile([128, KT * D], BF, tag="w0r")
                nc.scalar.dma_start(out=w0r[:, :], in_=w0r_d[:, :])

                # ---- h0 = x @ W0r -> [4][128 nodes, 768] bf16 ----
                # (emitted inside the e-loop at b==2 so the first e-blocks'
                # matmuls keep the et-DMA pipeline primed)
                h0 = [l0p.tile([128, D], BF, tag=f"h0_{m}", name=f"h0_{m}") for m in range(4)]

                def h0_compute(psh0):
                    for m in range(4):
                        pa = psh0.tile([128, 512], F32, tag="ph0a")
                        pb = psh0.tile([128, 256], F32, tag="ph0b")
                        for k in range(KT):
                            lhs = xT[:, k * N + 128 * m : k * N + 128 * (m + 1)]
                            nc.tensor.matmul(
                                pa[:, :], lhs, w0r[:, k * D : k * D + 512],
                                start=(k == 0), stop=(k == KT - 1),
                            )
                            nc.tensor.matmul(
                                pb[:, :], lhs, w0r[:, k * D + 512 : (k + 1) * D],
                                start=(k == 0), stop=(k == KT - 1),
                            )
                        nc.vector.tensor_copy(out=h0[m][:, 0:512], in_=pa[:, :])
                        nc.vector.tensor_copy(out=h0[m][:, 512:768], in_=pb[:, :])
                    if DEBUG:
                        h0f = l0p.tile([128, D], F32, tag="h0f")
                        nc.vector.tensor_copy(out=h0f[:, :], in_=h0[0][:, :])
                        nc.sync.dma_start(out=dbg["h0"][:, :], in_=h0f[:, :])

                # ---- e-pass: scores, L1 staging, L0 softmax, att0^T ----
                at0T = [
                    l0p.tile([128, NBLK * 128], BF, tag=f"at0T_{q}", name=f"at0T_{q}") for q in range(4)
                ]
                with (
                    tc.tile_pool(name="psv", bufs=2, space="PSUM") as psvp,
                    tc.tile_pool(name="pst", bufs=2, space="PSUM") as pstp,
                    tc.tile_pool(name="psx1", bufs=1, space="PSUM") as psx1,
                    tc.tile_pool(name="psh0", bufs=1, space="PSUM") as psh0,
                ):

                    def x1_chunk(k):
                        """x1 rows [32k, 32k+32) = elu(att0 @ h0), then AllGather."""
                        px1a = psx1.tile([32, 512], F32, tag="px1a")
                        px1b = psx1.tile([32, 256], F32, tag="px1b")
                        for h in range(H):
                            dsti = (
                                px1a[:, 64 * h : 64 * (h + 1)]
                                if h < 8
                                else px1b[:, 64 * (h - 8) : 64 * (h - 7)]
                            )
                            for q in range(4):
                                lhs = at0T[q][:, :].rearrange(
                                    "p (b c r) -> p b c r", b=NBLK, c=4
                                )[:, 8 * k : 8 * k + 8, :, h : h + 1]
                                nc.tensor.matmul(
                                    dsti, lhs, h0[q][:, 64 * h : 64 * (h + 1)],
                                    start=(q == 0), stop=(q == 3),
                                )
                        x1p = work.tile([32, D], F32, tag="x1p")
                        nc.vector.tensor_copy(out=x1p[:, 0:512], in_=px1a[:, :])
                        nc.vector.tensor_copy(out=x1p[:, 512:768], in_=px1b[:, :])
                        tmin = work.tile([32, D], F32, tag="tmin")
                        nc.vector.tensor_scalar(
                            out=tmin[:, :], in0=x1p[:, :], scalar1=0.0, scalar2=None,
                            op0=mybir.AluOpType.min,
                        )
                        texp = work.tile([32, D], F32, tag="texp")
                        nc.scalar.activation(texp[:, :], tmin[:, :], AF.Exp)
                        tmax = work.tile([32, D], F32, tag="tmax")
                        nc.vector.tensor_scalar(
                            out=tmax[:, :], in0=x1p[:, :], scalar1=0.0, scalar2=None,
                            op0=mybir.AluOpType.max,
                        )
                        x1bf = work.tile([32, D], BF, tag="x1bf")
                        nc.vector.scalar_tensor_tensor(
                            out=x1bf[:, :], in0=texp[:, :], scalar=-1.0, in1=tmax[:, :],
                            op0=ADD, op1=ADD,
                        )
                        if DEBUG:
                            x1f32 = work.tile([32, D], F32, tag="x1f32")
                            nc.vector.scalar_tensor_tensor(
                                out=x1f32[:, :], in0=texp[:, :], scalar=-1.0,
                                in1=tmax[:, :], op0=ADD, op1=ADD,
                            )
                            nc.sync.dma_start(
                                out=dbg["x1"][32 * k : 32 * (k + 1), :], in_=x1f32[:, :]
                            )
                        if k == 0:
                            nc.scalar.dma_start(out=agx_in[k][:, :], in_=x1bf[:, :])
                            nc.gpsimd.collective_compute(
                                "AllGather", mybir.AluOpType.bypass,
                                replica_groups=groups,
                                ins=[agx_in[k].ap().opt()], outs=[agx_out[k].ap().opt()],
                            )
                        else:
                            # replicate x1 chunk-1 into every dst's AllToAll
                            # block: AT_b doubles as the 2nd x1 AllGather.
                            for g in range(NCORES):
                                eng = nc.sync
                                eng.dma_start(
                                    out=at_in_b[
                                        g, NB * 2 * N : NB * 2 * N + 32 * D
                                    ].rearrange("(r f) -> r f", r=32),
                                    in_=x1bf[:, :],
                                )

                    for b in range(NBLK):
                        if b == 2:
                            h0_compute(psh0)
                        et = epool.tile([128, 24 * N], EDT, tag="etile")
                        nc.sync.dma_start(out=et[:, :], in_=eT_d[b])
                        ha = hapool.tile([128, N], BF, tag="ha0")
                        nc.scalar.dma_start(out=ha[:, :], in_=ha0_d[b])

                        psv = psvp.tile([128, N], F32, tag="psv")
                        for cc in range(4):
                            for kb in range(KT):
                                nc.tensor.matmul(
                                    psv[32 * cc : 32 * cc + 32, :],
                                    vw[:, 32 * kb : 32 * (kb + 1)],
                                    et[:, (cc * KT + kb) * N : (cc * KT + kb + 1) * N],
                                    start=(kb == 0), stop=(kb == KT - 1),
                                    tile_position=(0, 32 * cc),
                                )
                        # full scores (L0 rows 0:12, L1 rows 12:28 per cc-group)
                        sc0 = work.tile([128, N], BF, tag="sc0")
                        nc.vector.tensor_tensor(
                            out=sc0[:, :], in0=psv[:, :], in1=ha[:, :], op=ADD
                        )
                        if DEBUG and b == 0:
                            sc0f = work.tile([128, N], F32, tag="sc0f", bufs=1)
                            nc.vector.tensor_copy(out=sc0f[:, :], in_=sc0[:, :])
                            nc.sync.dma_start(out=dbg["sc0"][:, :], in_=sc0f[:, :])
                        # stage L1 rows to the AllToAll input buffer.
                        # NOTE: one DMA per cc-group — split-partition rearrange
                        # APs on SBUF tiles break tile dependency tracking.
                        for cc in range(4):
                            if b < SPLIT // 2:
                                dst = at_in_a1[:, 4 * b + cc, :, :]
                            elif b < SPLIT:
                                dst = at_in_a2[:, 4 * (b - SPLIT // 2) + cc, :, :]
                            else:
                                off = (4 * (b - SPLIT) + cc) * 2 * N
                                dst = at_in_b[:, off : off + 2 * N].rearrange(
                                    "g (s j) -> g s j", s=2, j=N
                                )
                            # gpsimd only BEFORE the first collective
                            # (collectives block that queue for their whole
                            # duration). Post-stream blocks go on the sync
                            # queue, which is idle once the e-DMAs finish.
                            if b >= SPLIT:
                                eng = nc.sync
                            elif cc < 2:
                                eng = nc.scalar
                            elif b < 5:
                                eng = nc.gpsimd
                            else:
                                eng = nc.scalar
                            eng.dma_start(
                                out=dst,
                                in_=sc0[32 * cc + 12 : 32 * cc + 28, :],
                            )
                        # layer-0 softmax (valid rows cc*32+[0:12); rest harmless)
                        # LeakyReLU on DVE as max(x, 0.2x) keeps the Activation
                        # engine Exp-only: no act-table reloads.
                        tn0 = work.tile([128, N], BF, tag="tn0")
                        nc.vector.tensor_scalar(
                            out=tn0[:, :], in0=sc0[:, :], scalar1=ALPHA, scalar2=None,
                            op0=MULT,
                        )
                        lr0 = work.tile([128, N], BF, tag="lr0")
                        nc.vector.tensor_tensor(
                            out=lr0[:, :], in0=sc0[:, :], in1=tn0[:, :],
                            op=mybir.AluOpType.max,
                        )
                        ex0 = work.tile([128, N], BF, tag="ex0")
                        z0 = work.tile([128, 1], F32, tag="z0")
                        nc.scalar.activation(
                            ex0[:, :], lr0[:, :], AF.Exp, accum_out=z0[:, :]
                        )
                        rz0 = work.tile([128, 1], F32, tag="rz0")
                        nc.vector.reciprocal(rz0[:, :], z0[:, :])
                        at0 = work.tile([128, N], BF, tag="at0")
                        nc.vector.tensor_scalar(
                            out=at0[:, :], in0=ex0[:, :], scalar1=rz0[:, :],
                            scalar2=None, op0=MULT,
                        )
                        if DEBUG and b == 0:
                            at0f = work.tile([128, N], F32, tag="at0f", bufs=1)
                            nc.vector.tensor_copy(out=at0f[:, :], in_=at0[:, :])
                            nc.sync.dma_start(out=dbg["at0"][:, :], in_=at0f[:, :])
                        for q in range(4):
                            pt = pstp.tile([128, 128], BF, tag="ptr")
                            nc.tensor.transpose(
                                pt[:, :], at0[:, 128 * q : 128 * (q + 1)], ident[:, :]
                            )
                            nc.vector.tensor_copy(
                                out=at0T[q][:, 128 * b : 128 * (b + 1)], in_=pt[:, :]
                            )
                        if b == SPLIT // 2 - 1:
                            nc.gpsimd.collective_compute(
                                "AllToAll", mybir.AluOpType.bypass,
                                replica_groups=groups,
                                ins=[at_in_a1.ap().opt()], outs=[at_out_a1.ap().opt()],
                            )
                        if b == SPLIT - 1:
                            nc.gpsimd.collective_compute(
                                "AllToAll", mybir.AluOpType.bypass,
                                replica_groups=groups,
                                ins=[at_in_a2.ap().opt()], outs=[at_out_a2.ap().opt()],
                            )
                        if b == 7:
                            x1_chunk(0)
                    x1_chunk(1)

                # second AllToAll chunk (ready at stream end)
                nc.gpsimd.collective_compute(
                    "AllToAll", mybir.AluOpType.bypass,
                    replica_groups=groups,
                    ins=[at_in_b.ap().opt()], outs=[at_out_b.ap().opt()],
                )
                with tc.tile_pool(name="x1fp", bufs=1) as x1fp:
                    x1f = [x1fp.tile([128, D], BF, tag=f"x1f_{m}", name=f"x1f_{m}") for m in range(4)]
                    for m in range(4):
                        # global row j = 64*src + 32*k + r -> partition 64*ds+32*k+r
                        for ds in range(2):
                            nc.sync.dma_start(
                                out=x1f[m][64 * ds : 64 * ds + 32, :],
                                in_=agx_out[0][
                                    32 * (2 * m + ds) : 32 * (2 * m + ds) + 32, :
                                ],
                            )
                            nc.sync.dma_start(
                                out=x1f[m][64 * ds + 32 : 64 * ds + 64, :],
                                in_=at_out_b[
                                    2 * m + ds, NB * 2 * N : NB * 2 * N + 32 * D
                                ].rearrange("(r f) -> r f", r=32),
                            )
                    with tc.tile_pool(name="psxt", bufs=2, space="PSUM") as psxt:
                        for m in range(4):
                            for k6 in range(KT):
                                pt = psxt.tile([128, 128], BF, tag="pxt")
                                nc.tensor.transpose(
                                    pt[:, :],
                                    x1f[m][:, 128 * k6 : 128 * (k6 + 1)],
                                    ident[:, :],
                                )
                                nc.vector.tensor_copy(
                                    out=x1T[
                                        :, N * k6 + 128 * m : N * k6 + 128 * (m + 1)
                                    ],
                                    in_=pt[:, :],
                                )

            # =================== tail: layer 1, head-sharded ===================
            with (
                tc.tile_pool(name="l1pers", bufs=1) as l1p,
                tc.tile_pool(name="l1work", bufs=2) as work,
                tc.tile_pool(name="scpool", bufs=3) as scpool,
                tc.tile_pool(name="at1pool", bufs=2) as at1pool,
            ):
                # s1sel = [src_A, src_B, dst_A, dst_B]^T [4, N]
                s1s = l1p.tile([4, N], BF, tag="s1s")
                srcT = l1p.tile([128, 8], BF, tag="srcT")  # [i, 2s] per g pair cols
                dm = [l1p.tile([128, N], BF, tag=f"dm_{s}", name=f"dm_{s}") for s in range(2)]
                with tc.tile_pool(name="pss1", bufs=1, space="PSUM") as pss1:
                    ps1 = pss1.tile([4, N], F32, tag="ps1")
                    for k in range(KT):
                        nc.tensor.matmul(
                            ps1[:, :], u1[:, 4 * k : 4 * (k + 1)],
                            x1T[:, N * k : N * (k + 1)],
                            start=(k == 0), stop=(k == KT - 1),
                        )
                    nc.vector.tensor_copy(out=s1s[:, :], in_=ps1[:, :])
                    if DEBUG:
                        s1f = work.tile([4, N], F32, tag="s1f", bufs=1)
                        nc.vector.tensor_copy(out=s1f[:, :], in_=ps1[:, :])
                        nc.sync.dma_start(out=dbg["s1sel"][:, :], in_=s1f[:, :])
                with tc.tile_pool(name="psdm", bufs=1, space="PSUM") as psdm:
                    # srcT[:, 2g+s] = s1sel[s, 128g:128(g+1)]
                    for g in range(4):
                        pt4 = psdm.tile([128, 2], BF, tag="pt4")
                        nc.tensor.transpose(
                            pt4[:, :], s1s[0:2, 128 * g : 128 * (g + 1)], ident[0:2, 0:2]
                        )
                        nc.vector.tensor_copy(
                            out=srcT[:, 2 * g : 2 * g + 2], in_=pt4[:, :]
                        )
                    # dm[s] = broadcast of dst row s over 128 partitions
                    for s in range(2):
                        pdm = psdm.tile([128, N], F32, tag="pdm")
                        nc.tensor.matmul(
                            pdm[:, :], ones[:, 128 * s : 128 * (s + 1)], s1s[0:4, :],
                            start=True, stop=True,
                        )
                        nc.vector.tensor_copy(out=dm[s][:, :], in_=pdm[:, :])

                # ---- h1 for my 2 heads: [4 jq][128, W1H] ----
                h1q = [l1p.tile([128, W1H], BF, tag=f"h1q_{q}", name=f"h1q_{q}") for q in range(4)]
                widths = [(0, 512), (512, 1024), (1024, 1536)]
                with tc.tile_pool(name="psh1", bufs=2, space="PSUM") as psh1:
                    for m in range(4):
                        ph1 = [
                            psh1.tile([128, 512], F32, tag="ph1a", name="ph1a"),
                            psh1.tile([128, 512], F32, tag="ph1b", name="ph1b"),
                            psh1.tile([128, 512], F32, tag="ph1c", name="ph1c"),
                        ]
                        for k in range(KT):
                            lhs = x1T[:, N * k + 128 * m : N * k + 128 * (m + 1)]
                            for t, (c0, c1) in enumerate(widths):
                                nc.tensor.matmul(
                                    ph1[t][:, 0 : c1 - c0], lhs,
                                    w1h[:, W1H * k + c0 : W1H * k + c1],
                                    start=(k == 0), stop=(k == KT - 1),
                                )
                        for t, (c0, c1) in enumerate(widths):
                            nc.scalar.copy(
                                out=h1q[m][:, c0:c1], in_=ph1[t][:, 0 : c1 - c0]
                            )

                # ---- per-igroup: softmax for both heads, att @ h1, partials ----
                with (
                    tc.tile_pool(name="pst1", bufs=2, space="PSUM") as pst1,
                    tc.tile_pool(name="pso", bufs=2, space="PSUM") as psop,
                ):
                    for g in range(4):
                        poa = psop.tile([128, 512], F32, tag="poa")
                        pob = psop.tile([128, 256], F32, tag="pob")
                        for s in range(2):
                            sct = scpool.tile([128, N], BF, tag="sct")
                            for ds in range(2):
                                nc.sync.dma_start(
                                    out=sct[64 * ds : 64 * ds + 2 * SPLIT, :],
                                    in_=at_out_a1[2 * g + ds, :, s, :],
                                )
                                nc.sync.dma_start(
                                    out=sct[64 * ds + 2 * SPLIT : 64 * ds + 4 * SPLIT, :],
                                    in_=at_out_a2[2 * g + ds, :, s, :],
                                )
                                nc.sync.dma_start(
                                    out=sct[64 * ds + 4 * SPLIT : 64 * (ds + 1), :],
                                    in_=at_out_b[
                                        2 * g + ds, 0 : NB * 2 * N
                                    ].rearrange("(i s j) -> i s j", s=2, j=N)[:, s, :],
                                )
                            sc1 = work.tile([128, N], BF, tag="sc1")
                            nc.vector.scalar_tensor_tensor(
                                out=sc1[:, :], in0=sct[:, :],
                                scalar=srcT[:, 2 * g + s : 2 * g + s + 1],
                                in1=dm[s][:, :], op0=ADD, op1=ADD,
                            )
                            if DEBUG and g == 0 and s == 0:
                                sctf = work.tile([128, N], F32, tag="sctf", bufs=1)
                                nc.vector.tensor_copy(out=sctf[:, :], in_=sc1[:, :])
                                nc.sync.dma_start(out=dbg["sct"][:, :], in_=sctf[:, :])
                            tn1 = work.tile([128, N], BF, tag="tn1")
                            nc.vector.tensor_scalar(
                                out=tn1[:, :], in0=sc1[:, :], scalar1=ALPHA,
                                scalar2=None, op0=MULT,
                            )
                            lr1 = work.tile([128, N], BF, tag="lr1")
                            nc.vector.tensor_tensor(
                                out=lr1[:, :], in0=sc1[:, :], in1=tn1[:, :],
                                op=mybir.AluOpType.max,
                            )
                            ex1 = work.tile([128, N], BF, tag="ex1")
                            z1 = work.tile([128, 1], F32, tag="z1")
                            nc.scalar.activation(
                                ex1[:, :], lr1[:, :], AF.Exp, accum_out=z1[:, :]
                            )
                            rz1 = work.tile([128, 1], F32, tag="rz1")
                            nc.vector.reciprocal(rz1[:, :], z1[:, :])
                            at1 = work.tile([128, N], BF, tag="at1")
                            if s == 1:
                                # head B is computed by two cores; halve it
                                nc.vector.tensor_scalar(
                                    out=at1[:, :], in0=ex1[:, :], scalar1=rz1[:, :],
                                    scalar2=0.5, op0=MULT, op1=MULT,
                                )
                            else:
                                nc.vector.tensor_scalar(
                                    out=at1[:, :], in0=ex1[:, :], scalar1=rz1[:, :],
                                    scalar2=None, op0=MULT,
                                )
                            if DEBUG and g == 0 and s == 0:
                                at1f = work.tile([128, N], F32, tag="at1f", bufs=1)
                                nc.vector.tensor_copy(out=at1f[:, :], in_=at1[:, :])
                                nc.sync.dma_start(out=dbg["at1"][:, :], in_=at1f[:, :])
                            at1T = at1pool.tile([128, 512], BF, tag="at1T")
                            for q in range(4):
                                pt = pst1.tile([128, 128], BF, tag="ptr1")
                                nc.tensor.transpose(
                                    pt[:, :], at1[:, 128 * q : 128 * (q + 1)], ident[:, :]
                                )
                                nc.vector.tensor_copy(
                                    out=at1T[:, 128 * q : 128 * (q + 1)], in_=pt[:, :]
                                )
                            for q in range(4):
                                lhsq = at1T[:, 128 * q : 128 * (q + 1)]
                                nc.tensor.matmul(
                                    poa[:, :], lhsq, h1q[q][:, D * s : D * s + 512],
                                    start=(s == 0 and q == 0), stop=(s == 1 and q == 3),
                                )
                                nc.tensor.matmul(
                                    pob[:, :], lhsq, h1q[q][:, D * s + 512 : D * (s + 1)],
                                    start=(s == 0 and q == 0), stop=(s == 1 and q == 3),
                                )
                        rsst = work.tile([128, D], F32, tag="rsst")
                        nc.vector.tensor_scalar(
                            out=rsst[:, 0:512], in0=poa[:, :], scalar1=1.0 / H,
                            scalar2=None, op0=MULT,
                        )
                        nc.vector.tensor_scalar(
                            out=rsst[:, 512:768], in0=pob[:, :], scalar1=1.0 / H,
                            scalar2=None, op0=MULT,
                        )
                        if DEBUG and g == 0:
                            nc.sync.dma_start(out=dbg["rsst"][:, :], in_=rsst[:, :])
                        nc.sync.dma_start(
                            out=rs_in[128 * g : 128 * (g + 1), :], in_=rsst[:, :]
                        )

                # ---- ReduceScatter partial outputs -> my 64 rows ----
                nc.gpsimd.collective_compute(
                    "ReduceScatter", ADD,
                    replica_groups=groups,
                    ins=[rs_in.ap().opt()], outs=[rs_out.ap().opt()],
                )
                opf = work.tile([64, D], F32, tag="opf", bufs=1)
                nc.sync.dma_start(out=opf[:, :], in_=rs_out[:, :])
                omin = work.tile([64, D], F32, tag="omin", bufs=1)
                nc.vector.tensor_scalar(
                    out=omin[:, :], in0=opf[:, :], scalar1=0.0, scalar2=None,
                    op0=mybir.AluOpType.min,
                )
                oexp = work.tile([64, D], F32, tag="oexp", bufs=1)
                nc.scalar.activation(oexp[:, :], omin[:, :], AF.Exp)
                omax = work.tile([64, D], F32, tag="omax", bufs=1)
                nc.vector.tensor_scalar(
                    out=omax[:, :], in0=opf[:, :], scalar1=0.0, scalar2=None,
                    op0=mybir.AluOpType.max,
                )
                ofin = work.tile([64, D], F32, tag="ofin", bufs=1)
                nc.vector.scalar_tensor_tensor(
                    out=ofin[:, :], in0=oexp[:, :], scalar=-1.0, in1=omax[:, :],
                    op0=ADD, op1=ADD,
                )
                nc.scalar.dma_start(out=out_d[:, :], in_=ofin[:, :])

    nc.compile()
    return nc


def _fold_weights(We, W, a, F_):
    We = We.astype(np.float64)
    W = W.astype(np.float64)
    a = a.astype(np.float64)
    a1, a2, a3 = a[:, :F_], a[:, F_ : 2 * F_], a[:, 2 * F_ :]
    v = np.einsum("hei,hif,hf->he", We, W, a3)
    usrc = np.einsum("hif,hf->hi", W, a1)
    udst = np.einsum("hif,hf->hi", W, a2)
    return v, usrc, udst


def _to_ktile(mat):
    """[768, C] -> [128, KT*C] with the KT k-tiles side by side."""
    k, c = mat.shape
    assert k == D
    return np.ascontiguousarray(
        mat.reshape(KT, 128, c).transpose(1, 0, 2).reshape(128, KT * c)
    )


def kernel(**inputs):
    global _COMPILED
    x = np.asarray(inputs["x"], dtype=np.float32)
    adj = np.asarray(inputs["adj"])
    e = np.asarray(inputs["e"], dtype=np.float32)
    W0 = np.asarray(inputs["W0"], dtype=np.float32)
    a0 = np.asarray(inputs["a0"], dtype=np.float32)
    W1 = np.asarray(inputs["W1"], dtype=np.float32)
    a1_ = np.asarray(inputs["a1"], dtype=np.float32)
    We0 = np.asarray(inputs["We0"], dtype=np.float32)
    We1 = np.asarray(inputs["We1"], dtype=np.float32)

    v0, _, _ = _fold_weights(We0, W0, a0, F0)
    v1, u1src, u1dst = _fold_weights(We1, W1, a1_, D)

    # V slot layout: 0-11 = layer-0 heads; 12+2g+s: s=0 -> head g, s=1 -> head 8+g//2
    V32 = np.zeros((D, 32), np.float64)
    V32[:, :12] = v0.T
    for g in range(NCORES):
        V32[:, 12 + 2 * g] = v1[g]
        V32[:, 12 + 2 * g + 1] = v1[8 + g // 2]
    v_bf = _to_ktile(V32.astype(np.float32)).astype(BF16)

    h0h = np.einsum("ni,hif->hnf", x.astype(np.float64), W0.astype(np.float64))
    s_src0 = np.einsum("hnf,hf->hn", h0h, a0[:, :F0].astype(np.float64))
    s_dst0 = np.einsum("hnf,hf->hn", h0h, a0[:, F0 : 2 * F0].astype(np.float64))
    maskadd = (adj.astype(np.float32) - 1.0) * 9e15                   # 0 or -9e15

    xT_bf = _to_ktile(np.ascontiguousarray(x.T)).astype(BF16)
    w0r_bf = _to_ktile(W0.transpose(1, 0, 2).reshape(D, H * F0)).astype(BF16)
    W1r = W1.transpose(1, 0, 2).reshape(D, H * D)
    ident = np.eye(128, dtype=np.float32).astype(BF16)
    onesel = np.zeros((4, 256), np.float32)
    onesel[2, 0:128] = 1.0
    onesel[3, 128:256] = 1.0
    onesel = onesel.astype(BF16)

    # block-major fp8 e layout: eb[c, b, p, cc, kb, j] = e[64c+4b+cc, j, 128kb+p]
    e8 = e.astype(ENP)                                   # [i, j, k]
    v8 = e8.reshape(NCORES, NBLK, 4, N, KT, 128)          # [c, b, cc, j, kb, p]
    eb = np.ascontiguousarray(v8.transpose(0, 1, 5, 2, 4, 3)).reshape(
        NCORES, NBLK, 128, 24 * N
    )

    in_maps = []
    for c in range(NCORES):
        hA = c
        hB = 8 + c // 2
        ha0 = np.zeros((NBLK, 128, N), dtype=np.float32)
        for b in range(NBLK):
            for cc in range(4):
                i = NPC * c + 4 * b + cc
                ha0[b, 32 * cc : 32 * cc + 12, :] = (
                    s_dst0 + s_src0[:, i : i + 1] + maskadd[i : i + 1, :]
                )
                ha0[b, 32 * cc + 12 : 32 * cc + 28, :] = maskadd[i : i + 1, :]
        w1h_bf = _to_ktile(
            np.ascontiguousarray(
                np.concatenate(
                    [W1r[:, D * hA : D * (hA + 1)], W1r[:, D * hB : D * (hB + 1)]],
                    axis=1,
                )
            )
        ).astype(BF16)
        u1sel = np.stack(
            [u1src[hA], u1src[hB], u1dst[hA], u1dst[hB]], axis=1
        ).astype(np.float32)                                # [768, 4]
        in_maps.append(
            {
                "eT": eb[c],
                "xT": xT_bf,
                "w0r": w0r_bf,
                "w1h": w1h_bf,
                "vw": v_bf,
                "u1sel": _to_ktile(u1sel).astype(BF16),
                "ha0": ha0.astype(BF16),
                "ident": ident,
                "onesel": onesel,
            }
        )

    if _COMPILED is None:
        _COMPILED = _build_nc()
    nc = _COMPILED

    res = run_bass_kernel_spmd(nc, in_maps, list(range(NCORES)))
    global _LAST_RESULTS
    _LAST_RESULTS = res.results
    out = np.concatenate([res.results[c]["out"] for c in range(NCORES)], axis=0)
    return out.astype(np.float32)


if __name__ == "__main__":
    import reference

    inputs = {k: np.asarray(v) for k, v in reference.setup_inputs().items()}
    got = kernel(**inputs)
    print("output shape:", got.shape, got.dtype)


# revision 56
# speedup vs baseline: 1.0584x; 1.0053x over previous
"""Bass/Trainium2 kernel for a 2-layer multi-head GAT (DocRE model).

Contract: kernel(**inputs) takes the FULL unsharded inputs as numpy arrays
and returns the FULL [512, 768] float32 output.

Sharding / dataflow (v2, collective-light):
- Layer 0 is row-sharded: each core streams its 64 rows of the edge tensor e
  once (block-major fp8-e3m4 layout) and computes BOTH layers' edge scores
  s_e = e @ V in a single pass (V holds 12 layer-0 head columns plus 16
  layer-1 "slot" columns = (dst core, slot) pairs).
- Layer-1 edge scores are AllToAll'd to a head-sharded layout DURING the
  e-stream (2 chunks; the first is hidden under the stream). Each core then
  owns 2 layer-1 heads: head A = core id (weight 1), head B = 8 + core//2
  (computed by two cores, each weighted 1/2).
- x1 is AllGathered once (small); h1 columns for the core's two heads are
  computed locally from the gathered x1 -> NO AllGather of the big h1.
- Per-head partial outputs (all 512 rows) are ReduceScatter'd (fp32 add) to
  give each core its 64 output rows. Total collective payload is ~25x
  smaller than the v1 AllGather-h1 scheme.
- Additive score pieces: layer-0 s_src+s_dst+mask and the layer-1 mask are
  folded into one host-precomputed per-block tile (ha0); layer-1 s_dst rides
  a PE outer-product broadcast, s_src rides the LeakyReLU activation bias.
"""

import sys

sys.path.insert(0, "/opt/trn_rl_repo")

import numpy as np
import ml_dtypes

from concourse import bass, bacc, mybir, tile
from concourse.bass_utils import run_bass_kernel_spmd

BF16 = ml_dtypes.bfloat16
FP8 = ml_dtypes.float8_e3m4

N = 512          # nodes
D = 768          # hidden
H = 12           # heads
F0 = 64          # layer-0 per-head dim
NCORES = 8
NPC = N // NCORES          # 64 local rows per core
NBLK = NPC // 4            # 16 blocks of 4 rows
ALPHA = 0.2
KT = D // 128              # 6 contraction tiles
W1H = 2 * D                # h1 columns per core (2 heads)
SPLIT = 14                 # e-blocks in the first AllToAll chunk
NEG = -9e15

E_FP8 = True               # stream e as fp8-e3m4 (else bf16)

F32 = mybir.dt.float32
BF = mybir.dt.bfloat16
EDT = mybir.dt.float8e3 if E_FP8 else mybir.dt.bfloat16
ENP = FP8 if E_FP8 else BF16
ADD = mybir.AluOpType.add
MULT = mybir.AluOpType.mult
AF = mybir.ActivationFunctionType

_COMPILED = None
DEBUG = False
SIM_SAFE = False  # replace Prelu (not in interp) with Relu for cost-model sims
_LAST_RESULTS = None


def _build_nc():
    nc = bacc.Bacc("TRN2", target_bir_lowering=False, num_devices=NCORES)
    lrelu_fn = AF.Relu if SIM_SAFE else AF.Prelu
    NA = 4 * SPLIT               # i-rows per src in AllToAll chunk a
    NB = NPC - NA                # i-rows per src in chunk b
    dbg = {}
    if DEBUG:
        dbg["sc0"] = nc.dram_tensor("dbg_sc0", [128, N], F32, kind="ExternalOutput")
        dbg["at0"] = nc.dram_tensor("dbg_at0", [128, N], F32, kind="ExternalOutput")
        dbg["at0T"] = nc.dram_tensor("dbg_at0T", [128, 128], F32, kind="ExternalOutput")
        dbg["x1p"] = nc.dram_tensor("dbg_x1p", [NPC, D], F32, kind="ExternalOutput")
        dbg["h0"] = nc.dram_tensor("dbg_h0", [128, D], F32, kind="ExternalOutput")
        dbg["x1"] = nc.dram_tensor("dbg_x1", [NPC, D], F32, kind="ExternalOutput")
        dbg["s1sel"] = nc.dram_tensor("dbg_s1sel", [4, N], F32, kind="ExternalOutput")
        dbg["sct"] = nc.dram_tensor("dbg_sct", [128, N], F32, kind="ExternalOutput")
        dbg["at1"] = nc.dram_tensor("dbg_at1", [128, N], F32, kind="ExternalOutput")
        dbg["h1q"] = nc.dram_tensor("dbg_h1q", [128, W1H], F32, kind="ExternalOutput")
        dbg["rsst"] = nc.dram_tensor("dbg_rsst", [128, D], F32, kind="ExternalOutput")

    eT_d = nc.dram_tensor("eT", [NBLK, 128, 24 * N], EDT, kind="ExternalInput")
    xT_d = nc.dram_tensor("xT", [128, KT * N], BF, kind="ExternalInput")
    w0r_d = nc.dram_tensor("w0r", [128, KT * D], BF, kind="ExternalInput")
    w1h_d = nc.dram_tensor("w1h", [128, KT * W1H], BF, kind="ExternalInput")
    v_d = nc.dram_tensor("vw", [128, KT * 32], BF, kind="ExternalInput")
    u1_d = nc.dram_tensor("u1sel", [128, KT * 4], BF, kind="ExternalInput")
    ha0_d = nc.dram_tensor("ha0", [NBLK, 128, N], BF, kind="ExternalInput")
    ident_d = nc.dram_tensor("ident", [128, 128], BF, kind="ExternalInput")
    # onesel[:, 128s:128(s+1)] is a [4,128] selector with row 2+s all-ones:
    # onesel[:, s].T @ s1s[0:4] broadcasts dst row s across 128 partitions.
    ones_d = nc.dram_tensor("onesel", [4, 256], BF, kind="ExternalInput")

    out_d = nc.dram_tensor("out", [NPC, D], F32, kind="ExternalOutput")

    NA1 = NA // 2
    at_in_a1 = nc.dram_tensor("at_in_a1", [NCORES, NA1, 2, N], BF)
    at_out_a1 = nc.dram_tensor("at_out_a1", [NCORES, NA1, 2, N], BF)
    at_in_a2 = nc.dram_tensor("at_in_a2", [NCORES, NA1, 2, N], BF)
    at_out_a2 = nc.dram_tensor("at_out_a2", [NCORES, NA1, 2, N], BF)
    agx_in = [
        nc.dram_tensor(f"agx_in{k}", [NPC // 2, D], BF) for k in range(2)
    ]
    agx_out = [
        nc.dram_tensor(f"agx_out{k}", [N // 2, D], BF, addr_space="Shared")
        for k in range(2)
    ]
    # chunk-b AllToAll payload per dst: [NB, 2, N] scores + [32, D] x1 chunk-1
    # (replicated to every dst -> the AllToAll doubles as the 2nd x1 AllGather)
    RB = NB * 2 * N + 32 * D
    at_in_b = nc.dram_tensor("at_in_b", [NCORES, RB], BF)
    at_out_b = nc.dram_tensor("at_out_b", [NCORES, RB], BF)
    rs_in = nc.dram_tensor("rs_in", [N, D], F32)
    rs_out = nc.dram_tensor("rs_out", [NPC, D], F32)

    groups = [list(range(NCORES))]

    with tile.TileContext(nc) as tc:
        with (
            tc.tile_pool(name="const", bufs=1) as constp,
            tc.tile_pool(name="pers", bufs=1) as pers,
            tc.tile_pool(name="hapool", bufs=3) as hapool,
        ):
            # const loads go on the DVE/PE DMA queues so the sync queue can
            # start streaming e-tiles immediately
            ident = constp.tile([128, 128], BF, tag="ident")
            nc.gpsimd.dma_start(out=ident[:, :], in_=ident_d[:, :])
            ones = constp.tile([4, 256], BF, tag="ones")
            nc.gpsimd.dma_start(out=ones[:, :], in_=ones_d[:, :])
            w1h = constp.tile([128, KT * W1H], BF, tag="w1h")
            nc.gpsimd.dma_start(out=w1h[:, :], in_=w1h_d[:, :])
            u1 = constp.tile([128, KT * 4], BF, tag="u1")
            nc.gpsimd.dma_start(out=u1[:, :], in_=u1_d[:, :])

            x1T = pers.tile([128, KT * N], BF, tag="x1T")

            # =================== phase A: e-stream + layer 0 ===================
            with (
                tc.tile_pool(name="l0pers", bufs=1) as l0p,
                tc.tile_pool(name="l0const", bufs=1) as l0c,
                tc.tile_pool(name="epool", bufs=3) as epool,
                tc.tile_pool(name="l0work", bufs=2) as work,
            ):
                vw = l0c.tile([128, KT * 32], BF, tag="vw")
                nc.sync.dma_start(out=vw[:, :], in_=v_d[:, :])
                xT = l0c.tile([128, KT * N], BF, tag="xT")
                nc.scalar.dma_start(out=xT[:, :], in_=xT_d[:, :])
                w0r = l0c.tile([128, KT * D], BF, tag="w0r")
                nc.scalar.dma_start(out=w0r[:, :], in_=w0r_d[:, :])

                # ---- h0 = x @ W0r -> [4][128 nodes, 768] bf16 ----
                # (emitted inside the e-loop at b==2 so the first e-blocks'
                # matmuls keep the et-DMA pipeline primed)
                h0 = [l0p.tile([128, D], BF, tag=f"h0_{m}", name=f"h0_{m}") for m in range(4)]

                def h0_compute(psh0):
                    for m in range(4):
                        pa = psh0.tile([128, 512], F32, tag="ph0a")
                        pb = psh0.tile([128, 256], F32, tag="ph0b")
                        for k in range(KT):
                            lhs = xT[:, k * N + 128 * m : k * N + 128 * (m + 1)]
                            nc.tensor.matmul(
                                pa[:, :], lhs, w0r[:, k * D : k * D + 512],
                                start=(k == 0), stop=(k == KT - 1),
                            )
                            nc.tensor.matmul(
                                pb[:, :], lhs, w0r[:, k * D + 512 : (k + 1) * D],
                                start=(k == 0), stop=(k == KT - 1),
                            )
                        nc.vector.tensor_copy(out=h0[m][:, 0:512], in_=pa[:, :])
                        nc.vector.tensor_copy(out=h0[m][:, 512:768], in_=pb[:, :])
                    if DEBUG:
                        h0f = l0p.tile([128, D], F32, tag="h0f")
                        nc.vector.tensor_copy(out=h0f[:, :], in_=h0[0][:, :])
                        nc.sync.dma_start(out=dbg["h0"][:, :], in_=h0f[:, :])

                # ---- e-pass: scores, L1 staging, L0 softmax, att0^T ----
                at0T = [
                    l0p.tile([128, NBLK * 128], BF, tag=f"at0T_{q}", name=f"at0T_{q}") for q in range(4)
                ]
                with (
                    tc.tile_pool(name="psv", bufs=2, space="PSUM") as psvp,
                    tc.tile_pool(name="pst", bufs=2, space="PSUM") as pstp,
                    tc.tile_pool(name="psx1", bufs=1, space="PSUM") as psx1,
                    tc.tile_pool(name="psh0", bufs=1, space="PSUM") as psh0,
                ):

                    def x1_chunk(k):
                        """x1 rows [32k, 32k+32) = elu(att0 @ h0), then AllGather."""
                        px1a = psx1.tile([32, 512], F32, tag="px1a")
                        px1b = psx1.tile([32, 256], F32, tag="px1b")
                        for h in range(H):
                            dsti = (
                                px1a[:, 64 * h : 64 * (h + 1)]
                                if h < 8
                                else px1b[:, 64 * (h - 8) : 64 * (h - 7)]
                            )
                            for q in range(4):
                                lhs = at0T[q][:, :].rearrange(
                                    "p (b c r) -> p b c r", b=NBLK, c=4
                                )[:, 8 * k : 8 * k + 8, :, h : h + 1]
                                nc.tensor.matmul(
                                    dsti, lhs, h0[q][:, 64 * h : 64 * (h + 1)],
                                    start=(q == 0), stop=(q == 3),
                                )
                        x1p = work.tile([32, D], F32, tag="x1p")
                        nc.vector.tensor_copy(out=x1p[:, 0:512], in_=px1a[:, :])
                        nc.vector.tensor_copy(out=x1p[:, 512:768], in_=px1b[:, :])
                        tmin = work.tile([32, D], F32, tag="tmin")
                        nc.vector.tensor_scalar(
                            out=tmin[:, :], in0=x1p[:, :], scalar1=0.0, scalar2=None,
                            op0=mybir.AluOpType.min,
                        )
                        texp = work.tile([32, D], F32, tag="texp")
                        nc.scalar.activation(texp[:, :], tmin[:, :], AF.Exp)
                        tmax = work.tile([32, D], F32, tag="tmax")
                        nc.vector.tensor_scalar(
                            out=tmax[:, :], in0=x1p[:, :], scalar1=0.0, scalar2=None,
                            op0=mybir.AluOpType.max,
                        )
                        x1bf = work.tile([32, D], BF, tag="x1bf")
                        nc.vector.scalar_tensor_tensor(
                            out=x1bf[:, :], in0=texp[:, :], scalar=-1.0, in1=tmax[:, :],
                            op0=ADD, op1=ADD,
                        )
                        if DEBUG:
                            x1f32 = work.tile([32, D], F32, tag="x1f32")
                            nc.vector.scalar_tensor_tensor(
                                out=x1f32[:, :], in0=texp[:, :], scalar=-1.0,
                                in1=tmax[:, :], op0=ADD, op1=ADD,
                            )
                            nc.sync.dma_start(
                                out=dbg["x1"][32 * k : 32 * (k + 1), :], in_=x1f32[:, :]
                            )
                        if k == 0:
                            nc.scalar.dma_start(out=agx_in[k][:, :], in_=x1bf[:, :])
                            nc.gpsimd.collective_compute(
                                "AllGather", mybir.AluOpType.bypass,
                                replica_groups=groups,
                                ins=[agx_in[k].ap().opt()], outs=[agx_out[k].ap().opt()],
                            )
                        else:
                            # replicate x1 chunk-1 into every dst's AllToAll
                            # block: AT_b doubles as the 2nd x1 AllGather.
                            for g in range(NCORES):
                                eng = nc.sync
                                eng.dma_start(
                                    out=at_in_b[
                                        g, NB * 2 * N : NB * 2 * N + 32 * D
                                    ].rearrange("(r f) -> r f", r=32),
                                    in_=x1bf[:, :],
                                )

                    for b in range(NBLK):
                        if b == 2:
                            h0_compute(psh0)
                        et = epool.tile([128, 24 * N], EDT, tag="etile")
                        nc.sync.dma_start(out=et[:, :], in_=eT_d[b])
                        ha = hapool.tile([128, N], BF, tag="ha0")
                        nc.scalar.dma_start(out=ha[:, :], in_=ha0_d[b])

                        psv = psvp.tile([128, N], F32, tag="psv")
                        for cc in range(4):
                            for kb in range(KT):
                                nc.tensor.matmul(
                                    psv[32 * cc : 32 * cc + 32, :],
                                    vw[:, 32 * kb : 32 * (kb + 1)],
                                    et[:, (cc * KT + kb) * N : (cc * KT + kb + 1) * N],
                                    start=(kb == 0), stop=(kb == KT - 1),
                                    tile_position=(0, 32 * cc),
                                )
                        # full scores (L0 rows 0:12, L1 rows 12:28 per cc-group)
                        sc0 = work.tile([128, N], BF, tag="sc0")
                        nc.vector.tensor_tensor(
                            out=sc0[:, :], in0=psv[:, :], in1=ha[:, :], op=ADD
                        )
                        if DEBUG and b == 0:
                            sc0f = work.tile([128, N], F32, tag="sc0f", bufs=1)
                            nc.vector.tensor_copy(out=sc0f[:, :], in_=sc0[:, :])
                            nc.sync.dma_start(out=dbg["sc0"][:, :], in_=sc0f[:, :])
                        # stage L1 rows to the AllToAll input buffer.
                        # NOTE: one DMA per cc-group — split-partition rearrange
                        # APs on SBUF tiles break tile dependency tracking.
                        for cc in range(4):
                            if b < SPLIT // 2:
                                dst = at_in_a1[:, 4 * b + cc, :, :]
                            elif b < SPLIT:
                                dst = at_in_a2[:, 4 * (b - SPLIT // 2) + cc, :, :]
                            else:
                                off = (4 * (b - SPLIT) + cc) * 2 * N
                                dst = at_in_b[:, off : off + 2 * N].rearrange(
                                    "g (s j) -> g s j", s=2, j=N
                                )
                            # gpsimd only BEFORE the first collective
                            # (collectives block that queue for their whole
                            # duration). Post-stream blocks go on the sync
                            # queue, which is idle once the e-DMAs finish.
                            if b >= SPLIT:
                                eng = nc.sync
                            elif cc < 2:
                                eng = nc.scalar
                            elif b < 5:
                                eng = nc.gpsimd
                            else:
                                eng = nc.scalar
                            eng.dma_start(
                                out=dst,
                                in_=sc0[32 * cc + 12 : 32 * cc + 28, :],
                            )
                        # layer-0 softmax (valid rows cc*32+[0:12); rest harmless)
                        # LeakyReLU on DVE as max(x, 0.2x) keeps the Activation
                        # engine Exp-only: no act-table reloads.
                        tn0 = work.tile([128, N], BF, tag="tn0")
                        nc.vector.tensor_scalar(
                            out=tn0[:, :], in0=sc0[:, :], scalar1=ALPHA, scalar2=None,
                            op0=MULT,
                        )
                        lr0 = work.tile([128, N], BF, tag="lr0")
                        nc.vector.tensor_tensor(
                            out=lr0[:, :], in0=sc0[:, :], in1=tn0[:, :],
                            op=mybir.AluOpType.max,
                        )
                        ex0 = work.tile([128, N], BF, tag="ex0")
                        z0 = work.tile([128, 1], F32, tag="z0")
                        nc.scalar.activation(
                            ex0[:, :], lr0[:, :], AF.Exp, accum_out=z0[:, :]
                        )
                        rz0 = work.tile([128, 1], F32, tag="rz0")
                        nc.vector.reciprocal(rz0[:, :], z0[:, :])
                        at0 = work.tile([128, N], BF, tag="at0")
                        nc.vector.tensor_scalar(
                            out=at0[:, :], in0=ex0[:, :], scalar1=rz0[:, :],
                            scalar2=None, op0=MULT,
                        )
                        if DEBUG and b == 0:
                            at0f = work.tile([128, N], F32, tag="at0f", bufs=1)
                            nc.vector.tensor_copy(out=at0f[:, :], in_=at0[:, :])
                            nc.sync.dma_start(out=dbg["at0"][:, :], in_=at0f[:, :])
                        for q in range(4):
                            pt = pstp.tile([128, 128], BF, tag="ptr")
                            nc.tensor.transpose(
                                pt[:, :], at0[:, 128 * q : 128 * (q + 1)], ident[:, :]
                            )
                            nc.vector.tensor_copy(
                                out=at0T[q][:, 128 * b : 128 * (b + 1)], in_=pt[:, :]
                            )
                        if b == SPLIT // 2 - 1:
                            nc.gpsimd.collective_compute(
                                "AllToAll", mybir.AluOpType.bypass,
                                replica_groups=groups,
                                ins=[at_in_a1.ap().opt()], outs=[at_out_a1.ap().opt()],
                            )
                        if b == SPLIT - 1:
                            nc.gpsimd.collective_compute(
                                "AllToAll", mybir.AluOpType.bypass,
                                replica_groups=groups,
                                ins=[at_in_a2.ap().opt()], outs=[at_out_a2.ap().opt()],
                            )
                        if b == 7:
                            x1_chunk(0)
                    x1_chunk(1)

                # second AllToAll chunk (ready at stream end)
                nc.gpsimd.collective_compute(
                    "AllToAll", mybir.AluOpType.bypass,
                    replica_groups=groups,
                    ins=[at_in_b.ap().opt()], outs=[at_out_b.ap().opt()],
                )
                with tc.tile_pool(name="x1fp", bufs=1) as x1fp:
                    x1f = [x1fp.tile([128, D], BF, tag=f"x1f_{m}", name=f"x1f_{m}") for m in range(4)]
                    for m in range(4):
                        # global row j = 64*src + 32*k + r -> partition 64*ds+32*k+r
                        for ds in range(2):
                            nc.sync.dma_start(
                                out=x1f[m][64 * ds : 64 * ds + 32, :],
                                in_=agx_out[0][
                                    32 * (2 * m + ds) : 32 * (2 * m + ds) + 32, :
                                ],
                            )
                            nc.sync.dma_start(
                                out=x1f[m][64 * ds + 32 : 64 * ds + 64, :],
                                in_=at_out_b[
                                    2 * m + ds, NB * 2 * N : NB * 2 * N + 32 * D
                                ].rearrange("(r f) -> r f", r=32),
                            )
                    with tc.tile_pool(name="psxt", bufs=2, space="PSUM") as psxt:
                        for m in range(4):
                            for k6 in range(KT):
                                pt = psxt.tile([128, 128], BF, tag="pxt")
                                nc.tensor.transpose(
                                    pt[:, :],
                                    x1f[m][:, 128 * k6 : 128 * (k6 + 1)],
                                    ident[:, :],
                                )
                                nc.vector.tensor_copy(
                                    out=x1T[
                                        :, N * k6 + 128 * m : N * k6 + 128 * (m + 1)
                                    ],
                                    in_=pt[:, :],
                                )

            # =================== tail: layer 1, head-sharded ===================
            with (
                tc.tile_pool(name="l1pers", bufs=1) as l1p,
                tc.tile_pool(name="l1work", bufs=2) as work,
                tc.tile_pool(name="scpool", bufs=3) as scpool,
                tc.tile_pool(name="at1pool", bufs=2) as at1pool,
            ):
                # s1sel = [src_A, src_B, dst_A, dst_B]^T [4, N]
                s1s = l1p.tile([4, N], BF, tag="s1s")
                srcT = l1p.tile([128, 8], BF, tag="srcT")  # [i, 2s] per g pair cols
                dm = [l1p.tile([128, N], BF, tag=f"dm_{s}", name=f"dm_{s}") for s in range(2)]
                with tc.tile_pool(name="pss1", bufs=1, space="PSUM") as pss1:
                    ps1 = pss1.tile([4, N], F32, tag="ps1")
                    for k in range(KT):
                        nc.tensor.matmul(
                            ps1[:, :], u1[:, 4 * k : 4 * (k + 1)],
                            x1T[:, N * k : N * (k + 1)],
                            start=(k == 0), stop=(k == KT - 1),
                        )
                    nc.vector.tensor_copy(out=s1s[:, :], in_=ps1[:, :])
                    if DEBUG:
                        s1f = work.tile([4, N], F32, tag="s1f", bufs=1)
                        nc.vector.tensor_copy(out=s1f[:, :], in_=ps1[:, :])
                        nc.sync.dma_start(out=dbg["s1sel"][:, :], in_=s1f[:, :])
                with tc.tile_pool(name="psdm", bufs=1, space="PSUM") as psdm:
                    # srcT[:, 2g+s] = s1sel[s, 128g:128(g+1)]
                    for g in range(4):
                        pt4 = psdm.tile([128, 2], BF, tag="pt4")
                        nc.tensor.transpose(
                            pt4[:, :], s1s[0:2, 128 * g : 128 * (g + 1)], ident[0:2, 0:2]
                        )
                        nc.vector.tensor_copy(
                            out=srcT[:, 2 * g : 2 * g + 2], in_=pt4[:, :]
                        )
                    # dm[s] = broadcast of dst row s over 128 partitions
                    for s in range(2):
                        pdm = psdm.tile([128, N], F32, tag="pdm")
                        nc.tensor.matmul(
                            pdm[:, :], ones[:, 128 * s : 128 * (s + 1)], s1s[0:4, :],
                            start=True, stop=True,
                        )
                        nc.vector.tensor_copy(out=dm[s][:, :], in_=pdm[:, :])

                # ---- h1 for my 2 heads: [4 jq][128, W1H] ----
                h1q = [l1p.tile([128, W1H], BF, tag=f"h1q_{q}", name=f"h1q_{q}") for q in range(4)]
                widths = [(0, 512), (512, 1024), (1024, 1536)]
                with tc.tile_pool(name="psh1", bufs=2, space="PSUM") as psh1:
                    for m in range(4):
                        ph1 = [
                            psh1.tile([128, 512], F32, tag="ph1a", name="ph1a"),
                            psh1.tile([128, 512], F32, tag="ph1b", name="ph1b"),
                            psh1.tile([128, 512], F32, tag="ph1c", name="ph1c"),
                        ]
                        for k in range(KT):
                            lhs = x1T[:, N * k + 128 * m : N * k + 128 * (m + 1)]
                            for t, (c0, c1) in enumerate(widths):
                                nc.tensor.matmul(
                                    ph1[t][:, 0 : c1 - c0], lhs,
                                    w1h[:, W1H * k + c0 : W1H * k + c1],
                                    start=(k == 0), stop=(k == KT - 1),
                                )
                        for t, (c0, c1) in enumerate(widths):
                            nc.scalar.copy(
                                out=h1q[m][:, c0:c1], in_=ph1[t][:, 0 : c1 - c0]
                            )

                # ---- per-igroup: softmax for both heads, att @ h1, partials ----
                with (
                    tc.tile_pool(name="pst1", bufs=2, space="PSUM") as pst1,
                    tc.tile_pool(name="pso", bufs=2, space="PSUM") as psop,
                ):
                    for g in range(4):
                        poa = psop.tile([128, 512], F32, tag="poa")
                        pob = psop.tile([128, 256], F32, tag="pob")
                        for s in range(2):
                            sct = scpool.tile([128, N], BF, tag="sct")
                            for ds in range(2):
                                nc.sync.dma_start(
                                    out=sct[64 * ds : 64 * ds + 2 * SPLIT, :],
                                    in_=at_out_a1[2 * g + ds, :, s, :],
                                )
                                nc.sync.dma_start(
                                    out=sct[64 * ds + 2 * SPLIT : 64 * ds + 4 * SPLIT, :],
                                    in_=at_out_a2[2 * g + ds, :, s, :],
                                )
                                nc.sync.dma_start(
                                    out=sct[64 * ds + 4 * SPLIT : 64 * (ds + 1), :],
                                    in_=at_out_b[
                                        2 * g + ds, 0 : NB * 2 * N
                                    ].rearrange("(i s j) -> i s j", s=2, j=N)[:, s, :],
                                )
                            sc1 = work.tile([128, N], BF, tag="sc1")
                            nc.vector.scalar_tensor_tensor(
                                out=sc1[:, :], in0=sct[:, :],
                                scalar=srcT[:, 2 * g + s : 2 * g + s + 1],
                                in1=dm[s][:, :], op0=ADD, op1=ADD,
                            )
                            if DEBUG and g == 0 and s == 0:
                                sctf = work.tile([128, N], F32, tag="sctf", bufs=1)
                                nc.vector.tensor_copy(out=sctf[:, :], in_=sc1[:, :])
                                nc.sync.dma_start(out=dbg["sct"][:, :], in_=sctf[:, :])
                            tn1 = work.tile([128, N], BF, tag="tn1")
                            nc.vector.tensor_scalar(
                                out=tn1[:, :], in0=sc1[:, :], scalar1=ALPHA,
                                scalar2=None, op0=MULT,
                            )
                            lr1 = work.tile([128, N], BF, tag="lr1")
                            nc.vector.tensor_tensor(
                                out=lr1[:, :], in0=sc1[:, :], in1=tn1[:, :],
                                op=mybir.AluOpType.max,
                            )
                            ex1 = work.tile([128, N], BF, tag="ex1")
                            z1 = work.tile([128, 1], F32, tag="z1")
                            nc.scalar.activation(
                                ex1[:, :], lr1[:, :], AF.Exp, accum_out=z1[:, :]
                            )
                            rz1 = work.tile([128, 1], F32, tag="rz1")
                            nc.vector.reciprocal(rz1[:, :], z1[:, :])
                            at1 = work.tile([128, N], BF, tag="at1")
                            if s == 1:
                                # head B is computed by two cores; halve it
                                nc.vector.tensor_scalar(
                                    out=at1[:, :], in0=ex1[:, :], scalar1=rz1[:, :],
                                    scalar2=0.5, op0=MULT, op1=MULT,
                                )
                            else:
                                nc.vector.tensor_scalar(
                                    out=at1[:, :], in0=ex1[:, :], scalar1=rz1[:, :],
                                    scalar2=None, op0=MULT,
                                )
                            if DEBUG and g == 0 and s == 0:
                                at1f = work.tile([128, N], F32, tag="at1f", bufs=1)
                                nc.vector.tensor_copy(out=at1f[:, :], in_=at1[:, :])
                                nc.sync.dma_start(out=dbg["at1"][:, :], in_=at1f[:, :])
                            at1T = at1pool.tile([128, 512], BF, tag="at1T")
                            for q in range(4):
                                pt = pst1.tile([128, 128], BF, tag="ptr1")
                                nc.tensor.transpose(
                                    pt[:, :], at1[:, 128 * q : 128 * (q + 1)], ident[:, :]
                                )
                                nc.vector.tensor_copy(
                                    out=at1T[:, 128 * q : 128 * (q + 1)], in_=pt[:, :]
                                )
                            for q in range(4):
                                lhsq = at1T[:, 128 * q : 128 * (q + 1)]
                                nc.tensor.matmul(
                                    poa[:, :], lhsq, h1q[q][:, D * s : D * s + 512],
                                    start=(s == 0 and q == 0), stop=(s == 1 and q == 3),
                                )
                                nc.tensor.matmul(
                                    pob[:, :], lhsq, h1q[q][:, D * s + 512 : D * (s + 1)],
                                    start=(s == 0 and q == 0), stop=(s == 1 and q == 3),
                                )
                        rsst = work.tile([128, D], F32, tag="rsst")
                        nc.vector.tensor_scalar(
                            out=rsst[:, 0:512], in0=poa[:, :], scalar1=1.0 / H,
                            scalar2=None, op0=MULT,
                        )
                        nc.vector.tensor_scalar(
                            out=rsst[:, 512:768], in0=pob[:, :], scalar1=1.0 / H,
                            scalar2=None, op0=MULT,
                        )
                        if DEBUG and g == 0:
                            nc.sync.dma_start(out=dbg["rsst"][:, :], in_=rsst[:, :])
                        nc.sync.dma_start(
                            out=rs_in[128 * g : 128 * (g + 1), :], in_=rsst[:, :]
                        )

                # ---- ReduceScatter partial outputs -> my 64 rows ----
                nc.gpsimd.collective_compute(
                    "ReduceScatter", ADD,
                    replica_groups=groups,
                    ins=[rs_in.ap().opt()], outs=[rs_out.ap().opt()],
                )
                opf = work.tile([64, D], F32, tag="opf", bufs=1)
                nc.sync.dma_start(out=opf[:, :], in_=rs_out[:, :])
                omin = work.tile([64, D], F32, tag="omin", bufs=1)
                nc.vector.tensor_scalar(
                    out=omin[:, :], in0=opf[:, :], scalar1=0.0, scalar2=None,
                    op0=mybir.AluOpType.min,
                )
                oexp = work.tile([64, D], F32, tag="oexp", bufs=1)
                nc.scalar.activation(oexp[:, :], omin[:, :], AF.Exp)
                omax = work.tile([64, D], F32, tag="omax", bufs=1)
                nc.vector.tensor_scalar(
                    out=omax[:, :], in0=opf[:, :], scalar1=0.0, scalar2=None,
                    op0=mybir.AluOpType.max,
                )
                ofin = work.tile([64, D], F32, tag="ofin", bufs=1)
                nc.vector.scalar_tensor_tensor(
                    out=ofin[:, :], in0=oexp[:, :], scalar=-1.0, in1=omax[:, :],
                    op0=ADD, op1=ADD,
                )
                nc.scalar.dma_start(out=out_d[:, :], in_=ofin[:, :])

    nc.compile()
    return nc


def _fold_weights(We, W, a, F_):
    We = We.astype(np.float64)
    W = W.astype(np.float64)
    a = a.astype(np.float64)
    a1, a2, a3 = a[:, :F_], a[:, F_ : 2 * F_], a[:, 2 * F_ :]
    v = np.einsum("hei,hif,hf->he", We, W, a3)
    usrc = np.einsum("hif,hf->hi", W, a1)
    udst = np.einsum("hif,hf->hi", W, a2)
    return v, usrc, udst


def _to_ktile(mat):
    """[768, C] -> [128, KT*C] with the KT k-tiles side by side."""
    k, c = mat.shape
    assert k == D
    return np.ascontiguousarray(
        mat.reshape(KT, 128, c).transpose(1, 0, 2).reshape(128, KT * c)
    )


def kernel(**inputs):
    global _COMPILED
    x = np.asarray(inputs["x"], dtype=np.float32)
    adj = np.asarray(inputs["adj"])
    e = np.asarray(inputs["e"], dtype=np.float32)
    W0 = np.asarray(inputs["W0"], dtype=np.float32)
    a0 = np.asarray(inputs["a0"], dtype=np.float32)
    W1 = np.asarray(inputs["W1"], dtype=np.float32)
    a1_ = np.asarray(inputs["a1"], dtype=np.float32)
    We0 = np.asarray(inputs["We0"], dtype=np.float32)
    We1 = np.asarray(inputs["We1"], dtype=np.float32)

    v0, _, _ = _fold_weights(We0, W0, a0, F0)
    v1, u1src, u1dst = _fold_weights(We1, W1, a1_, D)

    # V slot layout: 0-11 = layer-0 heads; 12+2g+s: s=0 -> head g, s=1 -> head 8+g//2
    V32 = np.zeros((D, 32), np.float64)
    V32[:, :12] = v0.T
    for g in range(NCORES):
        V32[:, 12 + 2 * g] = v1[g]
        V32[:, 12 + 2 * g + 1] = v1[8 + g // 2]
    v_bf = _to_ktile(V32.astype(np.float32)).astype(BF16)

    h0h = np.einsum("ni,hif->hnf", x.astype(np.float64), W0.astype(np.float64))
    s_src0 = np.einsum("hnf,hf->hn", h0h, a0[:, :F0].astype(np.float64))
    s_dst0 = np.einsum("hnf,hf->hn", h0h, a0[:, F0 : 2 * F0].astype(np.float64))
    maskadd = (adj.astype(np.float32) - 1.0) * 9e15                   # 0 or -9e15

    xT_bf = _to_ktile(np.ascontiguousarray(x.T)).astype(BF16)
    w0r_bf = _to_ktile(W0.transpose(1, 0, 2).reshape(D, H * F0)).astype(BF16)
    W1r = W1.transpose(1, 0, 2).reshape(D, H * D)
    ident = np.eye(128, dtype=np.float32).astype(BF16)
    onesel = np.zeros((4, 256), np.float32)
    onesel[2, 0:128] = 1.0
    onesel[3, 128:256] = 1.0
    onesel = onesel.astype(BF16)

    # block-major fp8 e layout: eb[c, b, p, cc, kb, j] = e[64c+4b+cc, j, 128kb+p]
    e8 = e.astype(ENP)                                   # [i, j, k]
    v8 = e8.reshape(NCORES, NBLK, 4, N, KT, 128)          # [c, b, cc, j, kb, p]
    eb = np.ascontiguousarray(v8.transpose(0, 1, 5, 2, 4, 3)).reshape(
        NCORES, NBLK, 128, 24 * N
    )

    in_maps = []
    for c in range(NCORES):
        hA = c
        hB = 8 + c // 2
        ha0 = np.zeros((NBLK, 128, N), dtype=np.float32)
        for b in range(NBLK):
            for cc in range(4):
                i = NPC * c + 4 * b + cc
                ha0[b, 32 * cc : 32 * cc + 12, :] = (
                    s_dst0 + s_src0[:, i : i + 1] + maskadd[i : i + 1, :]
                )
                ha0[b, 32 * cc + 12 : 32 * cc + 28, :] = maskadd[i : i + 1, :]
        w1h_bf = _to_ktile(
            np.ascontiguousarray(
                np.concatenate(
                    [W1r[:, D * hA : D * (hA + 1)], W1r[:, D * hB : D * (hB + 1)]],
                    axis=1,
                )
            )
        ).astype(BF16)
        u1sel = np.stack(
            [u1src[hA], u1src[hB], u1dst[hA], u1dst[hB]], axis=1
        ).astype(np.float32)                                # [768, 4]
        in_maps.append(
            {
                "eT": eb[c],
                "xT": xT_bf,
                "w0r": w0r_bf,
                "w1h": w1h_bf,
                "vw": v_bf,
                "u1sel": _to_ktile(u1sel).astype(BF16),
                "ha0": ha0.astype(BF16),
                "ident": ident,
                "onesel": onesel,
            }
        )

    if _COMPILED is None:
        _COMPILED = _build_nc()
    nc = _COMPILED

    res = run_bass_kernel_spmd(nc, in_maps, list(range(NCORES)))
    global _LAST_RESULTS
    _LAST_RESULTS = res.results
    out = np.concatenate([res.results[c]["out"] for c in range(NCORES)], axis=0)
    return out.astype(np.float32)


if __name__ == "__main__":
    import reference

    inputs = {k: np.asarray(v) for k, v in reference.setup_inputs().items()}
    got = kernel(**inputs)
    print("output shape:", got.shape, got.dtype)
